# revision 20
# baseline (speedup 1.0000x reference)
"""DND retrieval (episodic memory read) kernel for 8 Trainium2 NeuronCores.

Strategy: data-parallel over batch B=64 -> 8 envs per core. Per core:
  - q-side MLP chain: fp32 weights as the MOVING operand with the tiny
    [feat,8] activations stationary (avoids the very expensive fp32
    stationary-weight loads); natural-layout outputs are re-transposed
    between layers on the PE (cheap [8,128] blocks), biases added
    per-partition after the transpose. The wide Wq layer runs in bf16.
  - keys are cast f32->bf16 on DVE and transposed by the DMA xbar
    (dma_start_transpose) straight into the [k, l] layout - no PE time.
  - scores + value matmuls in bf16 with fp32 PSUM accumulation; all 8
    envs' scores accumulate into one [64, 512] PSUM bank via a
    zero-padded stationary operand.
  - rpe modulation folded into post-matmul scaling (it factors out of
    the k-contraction); validity mask built on-chip from iota + step.
  - softmax batched on a [64 (b*h), 1024 (l)] fp32 tile.
  - value_aggregator + read_memory chains: fp32 weights moving.
MLP weights are replicated per core and streamed from HBM.
"""
from contextlib import ExitStack

import numpy as np

import concourse.bass as bass
import concourse.tile as tile
from concourse import bacc, mybir
from concourse.bass_utils import run_bass_kernel_spmd
from concourse.masks import make_identity

F32 = mybir.dt.float32
BF16 = mybir.dt.bfloat16
AF = mybir.ActivationFunctionType
OP = mybir.AluOpType

L = 1024      # episode length (memory slots)
B = 64        # total batch
BL = 8        # batch per core
KD = 512      # key size
VD = 512      # value size
H = 8         # heads
MEMB = 256    # memory state embedding
SDIM = 512    # state dim
HID = 512
RIMQ = 512
LAT = KD - MEMB
NCORES = 8
LC = L // 128         # 8 l-chunks
KC = KD // 128        # 4 k-chunks
RSQK = 1.0 / np.sqrt(np.float32(KD))

_CACHE: dict = {}


def _emit(nc: bass.Bass, tc: tile.TileContext, ctx: ExitStack, io: dict):
    pool = ctx.enter_context(tc.tile_pool(name="main", bufs=1))
    kpool = ctx.enter_context(tc.tile_pool(name="keys", bufs=2))
    kbpool = ctx.enter_context(tc.tile_pool(name="keysb", bufs=2))
    vpool = ctx.enter_context(tc.tile_pool(name="vals", bufs=4))
    vbpool = ctx.enter_context(tc.tile_pool(name="valsb", bufs=6))
    ktpool = ctx.enter_context(tc.tile_pool(name="keysT", bufs=3))
    wpool = ctx.enter_context(tc.tile_pool(name="wstream", bufs=2))
    wbpool = ctx.enter_context(tc.tile_pool(name="wcast", bufs=4))
    wrpool = ctx.enter_context(tc.tile_pool(name="wres", bufs=8))
    psum = ctx.enter_context(tc.tile_pool(name="ps", bufs=4, space="PSUM"))
    spsum = ctx.enter_context(tc.tile_pool(name="ps2", bufs=3, space="PSUM"))

    ident = pool.tile([128, 128], F32)
    make_identity(nc, ident[:])
    identb = pool.tile([128, 128], BF16)
    make_identity(nc, identb[:])

    def bias_tile(name, nch):
        t = pool.tile([128, nch], F32, tag="b" + name)
        nc.sync.dma_start(t[:], io[name][:])
        return t

    # natural [8, N] psum -> bf16 sbuf -> per-128-block bf16 transpose ->
    # [128, 8] bf16 tiles with per-partition bias added
    def nat_to_T(nat_psum, n, b_tile, tag):
        natsb = pool.tile([BL, n], BF16, tag=f"nat{tag}")
        nc.scalar.copy(natsb[:], nat_psum[:])
        outs = []
        for j in range(n // 128):
            tp = psum.tile([128, BL], BF16, tag="sm")
            nc.tensor.transpose(tp[:], natsb[:, j * 128:(j + 1) * 128],
                                identb[0:BL, 0:BL])
            t = pool.tile([128, BL], BF16, tag=f"{tag}{j}")
            nc.vector.tensor_scalar(out=t[:], in0=tp[:],
                                    scalar1=b_tile[:, j:j + 1],
                                    scalar2=None, op0=OP.add)
            outs.append(t)
        return outs

    # bf16 layer: activations stationary [128,8] bf16 chunks, weights
    # streamed f32 in one DMA, cast to bf16 on DVE, used as moving operand
    def layer_bf16(xT_chunks, w_name, n_out, eng=None):
        nk = len(xT_chunks)
        w = wpool.tile([128, nk, n_out], F32, tag="Wstg")
        (eng or nc.sync).dma_start(
            w[:], io[w_name].rearrange("(f p) c -> p f c", p=128))
        wb = wbpool.tile([128, nk, n_out], BF16, tag="Wstgb")
        nc.vector.tensor_copy(wb[:], w[:])
        ps = spsum.tile([BL, n_out], F32, tag="sp")
        for k in range(nk):
            nc.tensor.matmul(ps[:], xT_chunks[k][:], wb[:, k, :],
                             start=(k == 0), stop=(k == nk - 1),
                             skip_group_check=True)
        return ps

    # ---------------- Phase A: q-side MLP ---------------------------------
    state_nat = pool.tile([BL, SDIM], F32)
    nc.sync.dma_start(state_nat[:], io["state"][:])
    lat_nat = pool.tile([BL, LAT], F32)
    nc.sync.dma_start(lat_nat[:], io["lat"][:])

    bst = bias_tile("b_state", 2)
    bcq1 = bias_tile("bcq1", 4)
    bcq2 = bias_tile("bcq2", 4)
    bq = bias_tile("bq", 32)

    def transp_in(src_ap, n_free_chunks, tag):
        outs = []
        for c in range(n_free_chunks):
            tp = psum.tile([128, BL], F32, tag="sm")
            nc.tensor.transpose(tp[:], src_ap[:, c * 128:(c + 1) * 128],
                                ident[0:BL, 0:BL])
            t = pool.tile([128, BL], BF16, tag=tag + str(c))
            nc.vector.tensor_copy(t[:], tp[:])
            outs.append(t)
        return outs

    stateT = transp_in(state_nat, SDIM // 128, "stT")   # 4 tiles
    latT = transp_in(lat_nat, LAT // 128, "laT")        # 2 tiles

    se_ps = layer_bf16(stateT, "W_state", MEMB)
    xT = nat_to_T(se_ps, MEMB, bst, "xT") + latT
    h1_ps = layer_bf16(xT, "Wcq1", HID, eng=nc.scalar)
    h1T = nat_to_T(h1_ps, HID, bcq1, "h1")
    qc_ps = layer_bf16(h1T, "Wcq2", KD)
    qcT = nat_to_T(qc_ps, KD, bcq2, "qc")

    # q = qc @ Wq (bf16, weights moving), scattered into zero-padded Qpad:
    # for (b, kc) the scores lhsT is Qpad[:, kc*512 + b*64 : +64] with the
    # (b', h) columns nonzero only at b'==b, so all 8 envs' scores matmuls
    # can accumulate into one [64, 512] PSUM bank.  Bias bq added after the
    # transpose (it is per q-column = per-partition there).
    Qpad = pool.tile([128, KC * BL * B], BF16)
    nc.gpsimd.memset(Qpad[:], 0.0)
    for jg in range(4):
        wts = []
        for k in range(KC):
            w = wpool.tile([128, 1024], F32, tag="Wq")
            eng = nc.sync if k % 2 == 0 else nc.scalar
            eng.dma_start(w[:], io["Wq"][k * 128:(k + 1) * 128,
                                         jg * 1024:(jg + 1) * 1024])
            wb = wbpool.tile([128, 1024], BF16, tag="Wqb")
            nc.vector.tensor_copy(wb[:], w[:])
            wts.append(wb)
        for hf in range(2):
            ng = jg * 2 + hf
            ps = spsum.tile([BL, 512], F32, tag="sp")
            for k in range(KC):
                nc.tensor.matmul(ps[:], qcT[k][:],
                                 wts[k][:, hf * 512:(hf + 1) * 512],
                                 start=(k == 0), stop=(k == KC - 1),
                                 skip_group_check=True)
            qnat = pool.tile([BL, 512], BF16, tag="qnat")
            nc.scalar.copy(qnat[:], ps[:])
            for jj in range(4):
                j = ng * 4 + jj
                h = j // KC
                kc = j % KC
                tp = psum.tile([128, BL], BF16, tag="sm")
                nc.tensor.transpose(tp[:], qnat[:, jj * 128:(jj + 1) * 128],
                                    identb[0:BL, 0:BL])
                base = kc * 512 + h
                nc.vector.tensor_scalar(
                    out=Qpad[:, base:base + (BL - 1) * 72 + 1:72],
                    in0=tp[:], scalar1=bq[:, j:j + 1],
                    scalar2=None, op0=OP.add)

    # -------- Wagg: stream early, cast to bf16 on idle GpSimd, residents --
    waggb = []
    for g in range(8):
        wstg = wpool.tile([128, 4, VD], F32, tag="Waggstg")
        eng = nc.sync if g % 2 == 0 else nc.scalar
        eng.dma_start(wstg[:], io["Wagg"][g * 512:(g + 1) * 512, :]
                      .rearrange("(f p) c -> p f c", p=128))
        wgb = wrpool.tile([128, 4, VD], BF16, tag="Waggb")
        nc.gpsimd.tensor_copy(wgb[:], wstg[:])
        waggb.append(wgb)

    # ---------------- Phase B: keys cast + xbar transpose + scores ---------
    # Per half-l-chunk (4 envs): one 1MB DMA, one cast to bf16, one xbar
    # transpose into [k, (b, kc), l] layout, then 16 N=128 scores matmuls.
    # Zero-padded lhsT -> every matmul writes the full [64, 512] bank; per
    # 128-col slice one accumulation group spans all (b, kc).
    S = pool.tile([B, L], F32)
    sp_half0 = spsum.tile([B, 512], F32, tag="sp")
    sp_half1 = spsum.tile([B, 512], F32, tag="sp")
    sp_halves = [sp_half0, sp_half1]
    for lc in range(LC):
        out_sl = sp_halves[lc // 4][:, (lc % 4) * 128:(lc % 4 + 1) * 128]
        for half in range(2):
            i = lc * 2 + half
            b0 = half * 4
            kbig = kpool.tile([128, 4, KD], F32, tag="kbig")
            eng = nc.sync if i % 2 == 0 else nc.scalar
            eng.dma_start(kbig[:],
                          io["keys"][lc * 128:(lc + 1) * 128, b0:b0 + 4, :])
            kb = kbpool.tile([128, 4, KD], BF16, tag="kb")
            if i % 2 == 0:
                nc.vector.tensor_copy(kb[:], kbig[:])
            else:
                nc.scalar.copy(kb[:], kbig[:])
            KTl = ktpool.tile([128, 4 * KC, 128], BF16, tag="KTl")
            eng.dma_start_transpose(KTl[:], kb[:])
            for bl in range(4):
                b = b0 + bl
                for kc in range(KC):
                    nc.tensor.matmul(out_sl,
                                     Qpad[:, kc * 512 + b * 64:
                                          kc * 512 + (b + 1) * 64],
                                     KTl[:, bl * KC + kc, :],
                                     start=(lc % 4 == 0 and half == 0
                                            and bl == 0 and kc == 0)
                                     or (bl == 0 and kc == 0 and half == 0),
                                     stop=(half == 1 and bl == 3
                                           and kc == KC - 1),
                                     skip_group_check=True)
    for lh in range(2):
        nc.vector.tensor_copy(S[:, lh * 512:(lh + 1) * 512], sp_halves[lh][:])

    # ---------------- Phase C: mask + softmax ------------------------------
    iot = pool.tile([B, L], F32)
    nc.gpsimd.iota(iot[:], pattern=[[1, L]], base=0, channel_multiplier=0,
                   allow_small_or_imprecise_dtypes=True)
    stept = pool.tile([B, 1], F32)
    nc.sync.dma_start(stept[:], io["step_rep"][:])
    valid = pool.tile([B, L], F32)
    nc.vector.tensor_scalar(out=valid[:], in0=iot[:], scalar1=stept[:, 0:1],
                            scalar2=None, op0=OP.is_lt)
    A = pool.tile([B, L], F32, tag="iot")
    nc.scalar.activation(A[:], valid[:], AF.Copy, bias=-1e30, scale=1e30)

    rpeT = pool.tile([BL, L], F32)
    for lc in range(LC):
        rp = pool.tile([128, BL], F32, tag="rp")
        nc.sync.dma_start(rp[:], io["rpe"][lc * 128:(lc + 1) * 128, :])
        tp = psum.tile([BL, 128], F32, tag="sm")
        nc.tensor.transpose(tp[:], rp[:], ident[:])
        nc.vector.tensor_copy(rpeT[:, lc * 128:(lc + 1) * 128], tp[:])
    selt = pool.tile([BL, B], F32)
    nc.sync.dma_start(selt[:], io["sel"][:])
    G = pool.tile([B, L], F32)
    for lh in range(2):
        gp = spsum.tile([B, 512], F32, tag="sp")
        nc.tensor.matmul(gp[:], selt[:], rpeT[:, lh * 512:(lh + 1) * 512],
                         start=True, stop=True)
        nc.vector.tensor_tensor(out=G[:, lh * 512:(lh + 1) * 512], in0=gp[:],
                                in1=valid[:, lh * 512:(lh + 1) * 512],
                                op=OP.mult)

    nc.vector.tensor_tensor(out=S[:], in0=S[:], in1=G[:], op=OP.mult)
    nc.vector.tensor_tensor(out=S[:], in0=S[:], in1=A[:], op=OP.add)
    negM = pool.tile([B, 1], F32)
    nc.vector.tensor_reduce(out=negM[:], in_=S[:], op=OP.max,
                            axis=mybir.AxisListType.X, negate=True)
    E = pool.tile([B, L], F32)
    Z = pool.tile([B, 1], F32)
    nc.scalar.activation(E[:], S[:], AF.Exp, bias=negM[:, 0:1], scale=1.0,
                         accum_out=Z[:, 0:1])
    R = pool.tile([B, 1], F32)
    nc.vector.reciprocal(R[:], Z[:])
    P = pool.tile([B, L], BF16)
    nc.vector.tensor_scalar(out=P[:], in0=E[:], scalar1=R[:, 0:1],
                            scalar2=None, op0=OP.mult)

    # ---------------- Phase D: prob transpose + value matmul ---------------
    PTs = []
    for lc in range(LC):
        PT = pool.tile([128, B], BF16, tag=f"PT{lc}")
        tpp = psum.tile([128, B], BF16, tag="sm")
        nc.tensor.transpose(tpp[:], P[:, lc * 128:(lc + 1) * 128],
                            identb[0:B, 0:B])
        nc.vector.tensor_copy(PT[:], tpp[:])
        PTs.append(PT)

    T = pool.tile([128, VD // 128, H, BL], BF16)
    for b in range(BL):
        rps = spsum.tile([BL, VD], F32, tag="sp")
        for lc in range(LC):
            i = b * LC + lc
            vn = vpool.tile([128, VD], F32, tag="vnat")
            veng = nc.sync if i % 2 == 0 else nc.scalar
            veng.dma_start(vn[:], io["vals"][lc * 128:(lc + 1) * 128, b, :])
            vb = vbpool.tile([128, VD], BF16, tag="vb")
            if i % 2 == 0:
                nc.vector.tensor_copy(vb[:], vn[:])
            else:
                nc.scalar.copy(vb[:], vn[:])
            nc.tensor.matmul(rps[:], PTs[lc][:, b * H:(b + 1) * H], vb[:],
                             start=(lc == 0), stop=(lc == LC - 1),
                             skip_group_check=True)
        rs = pool.tile([BL, VD], BF16, tag="rs")
        nc.scalar.copy(rs[:], rps[:])
        for vs in range(VD // 128):
            tr = psum.tile([128, BL], BF16, tag="sm")
            nc.tensor.transpose(tr[:], rs[:, vs * 128:(vs + 1) * 128],
                                identb[0:BL, 0:BL])
            nc.vector.tensor_copy(T[:, vs, :, b], tr[:])

    # ---------------- Phase E: output MLP chain ----------------------------
    bagg = bias_tile("bagg", 4)
    brk1 = bias_tile("brk1", 4)
    brv1 = bias_tile("brv1", 4)

    aggp = spsum.tile([BL, VD], F32, tag="sp")
    for c in range(32):
        g, f = c // 4, c % 4
        h = c // (VD // 128)
        vs = c % (VD // 128)
        nc.tensor.matmul(aggp[:], T[:, vs, h, :], waggb[g][:, f, :],
                         start=(c == 0), stop=(c == 31),
                         skip_group_check=True)
    AT = nat_to_T(aggp, VD, bagg, "AT")

    # final-layer biases broadcast to [8, 512] via K=1 matmul
    ones = pool.tile([1, BL], F32)
    nc.gpsimd.memset(ones[:], 1.0)

    def bias_bcast(name):
        brow = pool.tile([1, 512], F32, tag="br" + name)
        nc.sync.dma_start(brow[:], io[name][:])
        bb = psum.tile([BL, 512], F32, tag="sm")
        nc.tensor.matmul(bb[:], ones[:], brow[:], start=True, stop=True)
        bsb = pool.tile([BL, 512], F32, tag="bs" + name)
        nc.vector.tensor_copy(bsb[:], bb[:])
        return bsb

    bk2 = bias_bcast("brk2_flat")
    bv2 = bias_bcast("brv2_flat")

    hk_ps = layer_bf16(AT, "Wrk1", HID)
    hkT = nat_to_T(hk_ps, HID, brk1, "hk")
    ok_ps = layer_bf16(hkT, "Wrk2", RIMQ)
    hv_ps = layer_bf16(AT, "Wrv1", HID, eng=nc.scalar)
    hvT = nat_to_T(hv_ps, HID, brv1, "hv")
    ov_ps = layer_bf16(hvT, "Wrv2", VD, eng=nc.scalar)

    for name, ps_, bias_sb in (("out_key", ok_ps, bk2), ("out_val", ov_ps, bv2)):
        onat = pool.tile([BL, 512], F32, tag="o" + name)
        nc.vector.tensor_tensor(out=onat[:], in0=ps_[:], in1=bias_sb[:],
                                op=OP.add)
        nc.sync.dma_start(io[name][:], onat[:])


def _build():
    nc = bacc.Bacc("TRN2", target_bir_lowering=False, debug=False,
                   num_devices=NCORES)
    io = {}

    def din(name, shape):
        io[name] = nc.dram_tensor(name, shape, F32, kind="ExternalInput").ap()

    din("keys", [L, BL, KD])
    din("vals", [L, BL, VD])
    din("rpe", [L, BL])
    din("step_rep", [B, 1])
    din("state", [BL, SDIM])
    din("lat", [BL, LAT])
    din("sel", [BL, B])
    din("W_state", [SDIM, MEMB])
    din("b_state", [128, 2])
    din("Wcq1", [KD, HID])
    din("bcq1", [128, 4])
    din("Wcq2", [HID, KD])
    din("bcq2", [128, 4])
    din("Wq", [KD, H * KD])
    din("bq", [128, 32])
    din("Wagg", [H * VD, VD])
    din("bagg", [128, 4])
    din("Wrk1", [VD, HID])
    din("brk1", [128, 4])
    din("Wrk2", [HID, RIMQ])
    din("brk2_flat", [1, 512])
    din("Wrv1", [VD, HID])
    din("brv1", [128, 4])
    din("Wrv2", [HID, VD])
    din("brv2_flat", [1, 512])
    io["out_key"] = nc.dram_tensor("out_key", [BL, RIMQ], F32,
                                   kind="ExternalOutput").ap()
    io["out_val"] = nc.dram_tensor("out_val", [BL, VD], F32,
                                   kind="ExternalOutput").ap()

    with tile.TileContext(nc) as tc, ExitStack() as ctx:
        _emit(nc, tc, ctx, io)
    nc.compile()
    return nc


def _rsb(bias, nch):
    return np.ascontiguousarray(
        np.asarray(bias, np.float32).reshape(nch, 128).T)


def _shard(inputs):
    f = lambda x: np.asarray(x, np.float32)
    keys, vals, rpe = f(inputs["keys"]), f(inputs["vals"]), f(inputs["rpe_mod"])
    step = np.asarray(inputs["step"]).astype(np.float32)
    state, lat = f(inputs["state"]), f(inputs["task_inference_latent"])
    sel = np.ascontiguousarray(
        np.repeat(np.eye(BL, dtype=np.float32), BL, axis=1) * RSQK)
    shared = {
        "sel": sel,
        "W_state": f(inputs["W_state"]), "b_state": _rsb(inputs["b_state"], 2),
        "Wcq1": f(inputs["Wcq1"]), "bcq1": _rsb(inputs["bcq1"], 4),
        "Wcq2": f(inputs["Wcq2"]), "bcq2": _rsb(inputs["bcq2"], 4),
        "Wq": f(inputs["Wq"]), "bq": _rsb(inputs["bq"], 32),
        "Wagg": f(inputs["Wagg"]), "bagg": _rsb(inputs["bagg"], 4),
        "Wrk1": f(inputs["Wrk1"]), "brk1": _rsb(inputs["brk1"], 4),
        "Wrk2": f(inputs["Wrk2"]),
        "brk2_flat": np.ascontiguousarray(f(inputs["brk2"])[None, :]),
        "Wrv1": f(inputs["Wrv1"]), "brv1": _rsb(inputs["brv1"], 4),
        "Wrv2": f(inputs["Wrv2"]),
        "brv2_flat": np.ascontiguousarray(f(inputs["brv2"])[None, :]),
    }
    in_maps = []
    for m in range(NCORES):
        b0 = m * BL
        in_maps.append({
            "keys": np.ascontiguousarray(keys[:, b0:b0 + BL, :]),
            "vals": np.ascontiguousarray(vals[:, b0:b0 + BL, :]),
            "rpe": np.ascontiguousarray(rpe[:, b0:b0 + BL, 0]),
            "step_rep": np.ascontiguousarray(
                np.repeat(step[b0:b0 + BL], H)[:, None]),
            "state": np.ascontiguousarray(state[b0:b0 + BL]),
            "lat": np.ascontiguousarray(lat[b0:b0 + BL]),
            **shared,
        })
    return in_maps


def kernel(**inputs):
    nc = _CACHE.get("nc")
    if nc is None:
        nc = _CACHE["nc"] = _build()
    in_maps = _shard(inputs)
    res = run_bass_kernel_spmd(nc, in_maps, list(range(NCORES)),
                               **_CACHE.get("run_kwargs", {}))
    _CACHE["last_result"] = res
    ok = np.concatenate([res.results[m]["out_key"] for m in range(NCORES)], 0)
    ov = np.concatenate([res.results[m]["out_val"] for m in range(NCORES)], 0)
    return ok[:, None, :], ov[:, None, :]


# revision 22
# speedup vs baseline: 1.2806x; 1.2806x over previous
"""DND retrieval (episodic memory read) kernel for 8 Trainium2 NeuronCores.

Strategy: data-parallel over batch B=64 -> 8 envs per core. Per core:
  - q-side MLP chain: fp32 weights as the MOVING operand with the tiny
    [feat,8] activations stationary (avoids the very expensive fp32
    stationary-weight loads); natural-layout outputs are re-transposed
    between layers on the PE (cheap [8,128] blocks), biases added
    per-partition after the transpose. The wide Wq layer runs in bf16.
  - keys are cast f32->bf16 on DVE and transposed by the DMA xbar
    (dma_start_transpose) straight into the [k, l] layout - no PE time.
  - scores + value matmuls in bf16 with fp32 PSUM accumulation; all 8
    envs' scores accumulate into one [64, 512] PSUM bank via a
    zero-padded stationary operand.
  - rpe modulation folded into post-matmul scaling (it factors out of
    the k-contraction); validity mask built on-chip from iota + step.
  - softmax batched on a [64 (b*h), 1024 (l)] fp32 tile.
  - value_aggregator + read_memory chains: fp32 weights moving.
MLP weights are replicated per core and streamed from HBM.
"""
from contextlib import ExitStack

import numpy as np

import concourse.bass as bass
import concourse.tile as tile
from concourse import bacc, mybir
from concourse.bass_utils import run_bass_kernel_spmd
from concourse.masks import make_identity

F32 = mybir.dt.float32
BF16 = mybir.dt.bfloat16
AF = mybir.ActivationFunctionType
OP = mybir.AluOpType

L = 1024      # episode length (memory slots)
B = 64        # total batch
BL = 8        # batch per core
KD = 512      # key size
VD = 512      # value size
H = 8         # heads
MEMB = 256    # memory state embedding
SDIM = 512    # state dim
HID = 512
RIMQ = 512
LAT = KD - MEMB
NCORES = 8
LC = L // 128         # 8 l-chunks
KC = KD // 128        # 4 k-chunks
RSQK = 1.0 / np.sqrt(np.float32(KD))

_CACHE: dict = {}


def _emit(nc: bass.Bass, tc: tile.TileContext, ctx: ExitStack, io: dict):
    pool = ctx.enter_context(tc.tile_pool(name="main", bufs=1))
    kpool = ctx.enter_context(tc.tile_pool(name="keys", bufs=2))
    kbpool = ctx.enter_context(tc.tile_pool(name="keysb", bufs=2))
    vpool = ctx.enter_context(tc.tile_pool(name="vals", bufs=3))
    vbpool = ctx.enter_context(tc.tile_pool(name="valsb", bufs=3))
    wpool = ctx.enter_context(tc.tile_pool(name="wstream", bufs=2))
    wbpool = ctx.enter_context(tc.tile_pool(name="wcast", bufs=3))
    wrpool = ctx.enter_context(tc.tile_pool(name="wres", bufs=8))
    psum = ctx.enter_context(tc.tile_pool(name="ps", bufs=4, space="PSUM"))
    spsum = ctx.enter_context(tc.tile_pool(name="ps2", bufs=3, space="PSUM"))

    ident = pool.tile([128, 128], F32)
    make_identity(nc, ident[:])
    identb = pool.tile([128, 128], BF16)
    make_identity(nc, identb[:])

    def bias_tile(name, nch):
        t = pool.tile([128, nch], F32, tag="b" + name)
        nc.sync.dma_start(t[:], io[name][:])
        return t

    # natural [8, N] psum -> bf16 sbuf -> per-128-block bf16 transpose ->
    # [128, 8] bf16 tiles with per-partition bias added
    def nat_to_T(nat_psum, n, b_tile, tag):
        natsb = pool.tile([BL, n], BF16, tag=f"nat{tag}")
        nc.scalar.copy(natsb[:], nat_psum[:])
        outs = []
        for j in range(n // 128):
            tp = psum.tile([128, BL], BF16, tag="sm")
            nc.tensor.transpose(tp[:], natsb[:, j * 128:(j + 1) * 128],
                                identb[0:BL, 0:BL])
            t = pool.tile([128, BL], BF16, tag=f"{tag}{j}")
            nc.vector.tensor_scalar(out=t[:], in0=tp[:],
                                    scalar1=b_tile[:, j:j + 1],
                                    scalar2=None, op0=OP.add)
            outs.append(t)
        return outs

    # bf16 layer: activations stationary [128,8] bf16 chunks, weights
    # streamed f32 in one DMA, cast to bf16 on DVE, used as moving operand
    def layer_bf16(xT_chunks, w_name, n_out, eng=None):
        nk = len(xT_chunks)
        w = wpool.tile([128, nk, n_out], F32, tag="Wstg")
        (eng or nc.sync).dma_start(
            w[:], io[w_name].rearrange("(f p) c -> p f c", p=128))
        wb = wbpool.tile([128, nk, n_out], BF16, tag="Wstgb")
        nc.vector.tensor_copy(wb[:], w[:])
        ps = spsum.tile([BL, n_out], F32, tag="sp")
        for k in range(nk):
            nc.tensor.matmul(ps[:], xT_chunks[k][:], wb[:, k, :],
                             start=(k == 0), stop=(k == nk - 1),
                             skip_group_check=True)
        return ps

    # ---------------- Phase A: q-side MLP ---------------------------------
    state_nat = pool.tile([BL, SDIM], F32)
    nc.sync.dma_start(state_nat[:], io["state"][:])
    lat_nat = pool.tile([BL, LAT], F32)
    nc.sync.dma_start(lat_nat[:], io["lat"][:])

    bst = bias_tile("b_state", 2)
    bcq1 = bias_tile("bcq1", 4)
    bcq2 = bias_tile("bcq2", 4)
    bq = bias_tile("bq", 32)

    def transp_in(src_ap, n_free_chunks, tag):
        outs = []
        for c in range(n_free_chunks):
            tp = psum.tile([128, BL], F32, tag="sm")
            nc.tensor.transpose(tp[:], src_ap[:, c * 128:(c + 1) * 128],
                                ident[0:BL, 0:BL])
            t = pool.tile([128, BL], BF16, tag=tag + str(c))
            nc.vector.tensor_copy(t[:], tp[:])
            outs.append(t)
        return outs

    stateT = transp_in(state_nat, SDIM // 128, "stT")   # 4 tiles
    latT = transp_in(lat_nat, LAT // 128, "laT")        # 2 tiles

    se_ps = layer_bf16(stateT, "W_state", MEMB)
    xT = nat_to_T(se_ps, MEMB, bst, "xT") + latT
    h1_ps = layer_bf16(xT, "Wcq1", HID, eng=nc.scalar)
    h1T = nat_to_T(h1_ps, HID, bcq1, "h1")
    qc_ps = layer_bf16(h1T, "Wcq2", KD)
    qcT = nat_to_T(qc_ps, KD, bcq2, "qc")

    # q = qc @ Wq (bf16, weights moving), scattered into zero-padded Qpad:
    # for (b, kc) the scores lhsT is Qpad[:, kc*512 + b*64 : +64] with the
    # (b', h) columns nonzero only at b'==b, so all 8 envs' scores matmuls
    # can accumulate into one [64, 512] PSUM bank.  Bias bq added after the
    # transpose (it is per q-column = per-partition there).
    Qpad = pool.tile([128, KC * BL * B], BF16)
    nc.gpsimd.memset(Qpad[:], 0.0)
    for jg in range(4):
        wts = []
        for k in range(KC):
            w = wpool.tile([128, 1024], F32, tag="Wq")
            eng = nc.sync if k % 2 == 0 else nc.scalar
            eng.dma_start(w[:], io["Wq"][k * 128:(k + 1) * 128,
                                         jg * 1024:(jg + 1) * 1024])
            wb = wbpool.tile([128, 1024], BF16, tag="Wqb")
            nc.vector.tensor_copy(wb[:], w[:])
            wts.append(wb)
        for hf in range(2):
            ng = jg * 2 + hf
            ps = spsum.tile([BL, 512], F32, tag="sp")
            for k in range(KC):
                nc.tensor.matmul(ps[:], qcT[k][:],
                                 wts[k][:, hf * 512:(hf + 1) * 512],
                                 start=(k == 0), stop=(k == KC - 1),
                                 skip_group_check=True)
            qnat = pool.tile([BL, 512], BF16, tag="qnat")
            nc.scalar.copy(qnat[:], ps[:])
            for jj in range(4):
                j = ng * 4 + jj
                h = j // KC
                kc = j % KC
                tp = psum.tile([128, BL], BF16, tag="sm")
                nc.tensor.transpose(tp[:], qnat[:, jj * 128:(jj + 1) * 128],
                                    identb[0:BL, 0:BL])
                base = kc * 512 + h
                nc.vector.tensor_scalar(
                    out=Qpad[:, base:base + (BL - 1) * 72 + 1:72],
                    in0=tp[:], scalar1=bq[:, j:j + 1],
                    scalar2=None, op0=OP.add)

    # -------- Wagg: stream early, cast to bf16 on idle GpSimd, residents --
    waggb = []
    for g in range(8):
        wstg = wpool.tile([128, 4, VD], F32, tag="Waggstg")
        eng = nc.sync if g % 2 == 0 else nc.scalar
        eng.dma_start(wstg[:], io["Wagg"][g * 512:(g + 1) * 512, :]
                      .rearrange("(f p) c -> p f c", p=128))
        wgb = wrpool.tile([128, 4, VD], BF16, tag="Waggb")
        nc.gpsimd.tensor_copy(wgb[:], wstg[:])
        waggb.append(wgb)

    # ---------------- Phase B: keys (pre-transposed [K, B, L]) + scores ----
    # Keys arrive in [k, b, l] layout (relayout chosen at shard time), so
    # each [128, 4, 1024] f32 DMA slice is cast to bf16 and fed straight to
    # the PE as the moving operand.  Zero-padded lhsT -> every matmul
    # writes the full [64, 512] bank; one accumulation group per lh bank
    # spans all (kc, b).
    S = pool.tile([B, L], F32)
    sp_half0 = spsum.tile([B, 512], F32, tag="sp")
    sp_half1 = spsum.tile([B, 512], F32, tag="sp")
    sp_halves = [sp_half0, sp_half1]
    for kc in range(KC):
        for half in range(2):
            i = kc * 2 + half
            b0 = half * 4
            ktn = kpool.tile([128, 4, L], F32, tag="ktn")
            eng = nc.sync if i % 2 == 0 else nc.scalar
            eng.dma_start(ktn[:],
                          io["keysT"][kc * 128:(kc + 1) * 128, b0:b0 + 4, :])
            ktb = kbpool.tile([128, 4, L], BF16, tag="ktb")
            if i % 2 == 0:
                nc.vector.tensor_copy(ktb[:], ktn[:])
            else:
                nc.scalar.copy(ktb[:], ktn[:])
            for bl in range(4):
                b = b0 + bl
                for lh in range(2):
                    nc.tensor.matmul(sp_halves[lh][:],
                                     Qpad[:, kc * 512 + b * 64:
                                          kc * 512 + (b + 1) * 64],
                                     ktb[:, bl, lh * 512:(lh + 1) * 512],
                                     start=(kc == 0 and half == 0
                                            and bl == 0),
                                     stop=(kc == KC - 1 and half == 1
                                           and bl == 3),
                                     skip_group_check=True)
    for lh in range(2):
        nc.vector.tensor_copy(S[:, lh * 512:(lh + 1) * 512], sp_halves[lh][:])

    # ---------------- Phase C: mask + softmax ------------------------------
    iot = pool.tile([B, L], F32)
    nc.gpsimd.iota(iot[:], pattern=[[1, L]], base=0, channel_multiplier=0,
                   allow_small_or_imprecise_dtypes=True)
    stept = pool.tile([B, 1], F32)
    nc.sync.dma_start(stept[:], io["step_rep"][:])
    valid = pool.tile([B, L], F32)
    nc.vector.tensor_scalar(out=valid[:], in0=iot[:], scalar1=stept[:, 0:1],
                            scalar2=None, op0=OP.is_lt)
    A = pool.tile([B, L], F32, tag="iot")
    nc.scalar.activation(A[:], valid[:], AF.Copy, bias=-1e30, scale=1e30)

    rpeT = pool.tile([BL, L], F32)
    for lc in range(LC):
        rp = pool.tile([128, BL], F32, tag="rp")
        nc.sync.dma_start(rp[:], io["rpe"][lc * 128:(lc + 1) * 128, :])
        tp = psum.tile([BL, 128], F32, tag="sm")
        nc.tensor.transpose(tp[:], rp[:], ident[:])
        nc.vector.tensor_copy(rpeT[:, lc * 128:(lc + 1) * 128], tp[:])
    selt = pool.tile([BL, B], F32)
    nc.sync.dma_start(selt[:], io["sel"][:])
    G = pool.tile([B, L], F32)
    for lh in range(2):
        gp = spsum.tile([B, 512], F32, tag="sp")
        nc.tensor.matmul(gp[:], selt[:], rpeT[:, lh * 512:(lh + 1) * 512],
                         start=True, stop=True)
        nc.vector.tensor_tensor(out=G[:, lh * 512:(lh + 1) * 512], in0=gp[:],
                                in1=valid[:, lh * 512:(lh + 1) * 512],
                                op=OP.mult)

    nc.vector.tensor_tensor(out=S[:], in0=S[:], in1=G[:], op=OP.mult)
    nc.vector.tensor_tensor(out=S[:], in0=S[:], in1=A[:], op=OP.add)
    negM = pool.tile([B, 1], F32)
    nc.vector.tensor_reduce(out=negM[:], in_=S[:], op=OP.max,
                            axis=mybir.AxisListType.X, negate=True)
    E = pool.tile([B, L], F32, tag="G")
    Z = pool.tile([B, 1], F32)
    nc.scalar.activation(E[:], S[:], AF.Exp, bias=negM[:, 0:1], scale=1.0,
                         accum_out=Z[:, 0:1])
    R = pool.tile([B, 1], F32)
    nc.vector.reciprocal(R[:], Z[:])
    P = pool.tile([B, L], BF16, tag="rpeT")
    nc.vector.tensor_scalar(out=P[:], in0=E[:], scalar1=R[:, 0:1],
                            scalar2=None, op0=OP.mult)

    # ---------------- Phase D: prob transpose + value matmul ---------------
    PTs = []
    for lc in range(LC):
        PT = pool.tile([128, B], BF16, tag=f"PT{lc}")
        tpp = psum.tile([128, B], BF16, tag="sm")
        nc.tensor.transpose(tpp[:], P[:, lc * 128:(lc + 1) * 128],
                            identb[0:B, 0:B])
        nc.vector.tensor_copy(PT[:], tpp[:])
        PTs.append(PT)

    T = pool.tile([128, VD // 128, H, BL], BF16)
    for b in range(BL):
        rps = spsum.tile([BL, VD], F32, tag="sp")
        for lq in range(LC // 2):
            i = b * (LC // 2) + lq
            vn = vpool.tile([128, 2, VD], F32, tag="vnat")
            veng = nc.sync if i % 2 == 0 else nc.scalar
            veng.dma_start(
                vn[:], io["vals"][lq * 256:(lq + 1) * 256, b, :]
                .rearrange("(f p) c -> p f c", p=128))
            vb = vbpool.tile([128, 2, VD], BF16, tag="vb")
            if i % 2 == 0:
                nc.vector.tensor_copy(vb[:], vn[:])
            else:
                nc.scalar.copy(vb[:], vn[:])
            for f in range(2):
                lc = lq * 2 + f
                nc.tensor.matmul(rps[:], PTs[lc][:, b * H:(b + 1) * H],
                                 vb[:, f, :],
                                 start=(lc == 0), stop=(lc == LC - 1),
                                 skip_group_check=True)
        rs = pool.tile([BL, VD], BF16, tag="rs")
        nc.scalar.copy(rs[:], rps[:])
        for vs in range(VD // 128):
            tr = psum.tile([128, BL], BF16, tag="sm")
            nc.tensor.transpose(tr[:], rs[:, vs * 128:(vs + 1) * 128],
                                identb[0:BL, 0:BL])
            nc.vector.tensor_copy(T[:, vs, :, b], tr[:])

    # ---------------- Phase E: output MLP chain ----------------------------
    bagg = bias_tile("bagg", 4)
    brk1 = bias_tile("brk1", 4)
    brv1 = bias_tile("brv1", 4)

    aggp = spsum.tile([BL, VD], F32, tag="sp")
    for c in range(32):
        g, f = c // 4, c % 4
        h = c // (VD // 128)
        vs = c % (VD // 128)
        nc.tensor.matmul(aggp[:], T[:, vs, h, :], waggb[g][:, f, :],
                         start=(c == 0), stop=(c == 31),
                         skip_group_check=True)
    AT = nat_to_T(aggp, VD, bagg, "AT")

    # final-layer biases broadcast to [8, 512] via K=1 matmul
    ones = pool.tile([1, BL], F32)
    nc.gpsimd.memset(ones[:], 1.0)

    def bias_bcast(name):
        brow = pool.tile([1, 512], F32, tag="br" + name)
        nc.sync.dma_start(brow[:], io[name][:])
        bb = psum.tile([BL, 512], F32, tag="sm")
        nc.tensor.matmul(bb[:], ones[:], brow[:], start=True, stop=True)
        bsb = pool.tile([BL, 512], F32, tag="bs" + name)
        nc.vector.tensor_copy(bsb[:], bb[:])
        return bsb

    bk2 = bias_bcast("brk2_flat")
    bv2 = bias_bcast("brv2_flat")

    hk_ps = layer_bf16(AT, "Wrk1", HID)
    hkT = nat_to_T(hk_ps, HID, brk1, "hk")
    ok_ps = layer_bf16(hkT, "Wrk2", RIMQ)
    hv_ps = layer_bf16(AT, "Wrv1", HID, eng=nc.scalar)
    hvT = nat_to_T(hv_ps, HID, brv1, "hv")
    ov_ps = layer_bf16(hvT, "Wrv2", VD, eng=nc.scalar)

    for name, ps_, bias_sb in (("out_key", ok_ps, bk2), ("out_val", ov_ps, bv2)):
        onat = pool.tile([BL, 512], F32, tag="o" + name)
        nc.vector.tensor_tensor(out=onat[:], in0=ps_[:], in1=bias_sb[:],
                                op=OP.add)
        nc.sync.dma_start(io[name][:], onat[:])


def _build():
    nc = bacc.Bacc("TRN2", target_bir_lowering=False, debug=False,
                   num_devices=NCORES)
    io = {}

    def din(name, shape):
        io[name] = nc.dram_tensor(name, shape, F32, kind="ExternalInput").ap()

    din("keysT", [KD, BL, L])
    din("vals", [L, BL, VD])
    din("rpe", [L, BL])
    din("step_rep", [B, 1])
    din("state", [BL, SDIM])
    din("lat", [BL, LAT])
    din("sel", [BL, B])
    din("W_state", [SDIM, MEMB])
    din("b_state", [128, 2])
    din("Wcq1", [KD, HID])
    din("bcq1", [128, 4])
    din("Wcq2", [HID, KD])
    din("bcq2", [128, 4])
    din("Wq", [KD, H * KD])
    din("bq", [128, 32])
    din("Wagg", [H * VD, VD])
    din("bagg", [128, 4])
    din("Wrk1", [VD, HID])
    din("brk1", [128, 4])
    din("Wrk2", [HID, RIMQ])
    din("brk2_flat", [1, 512])
    din("Wrv1", [VD, HID])
    din("brv1", [128, 4])
    din("Wrv2", [HID, VD])
    din("brv2_flat", [1, 512])
    io["out_key"] = nc.dram_tensor("out_key", [BL, RIMQ], F32,
                                   kind="ExternalOutput").ap()
    io["out_val"] = nc.dram_tensor("out_val", [BL, VD], F32,
                                   kind="ExternalOutput").ap()

    with tile.TileContext(nc) as tc, ExitStack() as ctx:
        _emit(nc, tc, ctx, io)
    nc.compile()
    return nc


def _rsb(bias, nch):
    return np.ascontiguousarray(
        np.asarray(bias, np.float32).reshape(nch, 128).T)


def _shard(inputs):
    f = lambda x: np.asarray(x, np.float32)
    keys, vals, rpe = f(inputs["keys"]), f(inputs["vals"]), f(inputs["rpe_mod"])
    step = np.asarray(inputs["step"]).astype(np.float32)
    state, lat = f(inputs["state"]), f(inputs["task_inference_latent"])
    sel = np.ascontiguousarray(
        np.repeat(np.eye(BL, dtype=np.float32), BL, axis=1) * RSQK)
    shared = {
        "sel": sel,
        "W_state": f(inputs["W_state"]), "b_state": _rsb(inputs["b_state"], 2),
        "Wcq1": f(inputs["Wcq1"]), "bcq1": _rsb(inputs["bcq1"], 4),
        "Wcq2": f(inputs["Wcq2"]), "bcq2": _rsb(inputs["bcq2"], 4),
        "Wq": f(inputs["Wq"]), "bq": _rsb(inputs["bq"], 32),
        "Wagg": f(inputs["Wagg"]), "bagg": _rsb(inputs["bagg"], 4),
        "Wrk1": f(inputs["Wrk1"]), "brk1": _rsb(inputs["brk1"], 4),
        "Wrk2": f(inputs["Wrk2"]),
        "brk2_flat": np.ascontiguousarray(f(inputs["brk2"])[None, :]),
        "Wrv1": f(inputs["Wrv1"]), "brv1": _rsb(inputs["brv1"], 4),
        "Wrv2": f(inputs["Wrv2"]),
        "brv2_flat": np.ascontiguousarray(f(inputs["brv2"])[None, :]),
    }
    in_maps = []
    for m in range(NCORES):
        b0 = m * BL
        in_maps.append({
            "keysT": np.ascontiguousarray(
                keys[:, b0:b0 + BL, :].transpose(2, 1, 0)),
            "vals": np.ascontiguousarray(vals[:, b0:b0 + BL, :]),
            "rpe": np.ascontiguousarray(rpe[:, b0:b0 + BL, 0]),
            "step_rep": np.ascontiguousarray(
                np.repeat(step[b0:b0 + BL], H)[:, None]),
            "state": np.ascontiguousarray(state[b0:b0 + BL]),
            "lat": np.ascontiguousarray(lat[b0:b0 + BL]),
            **shared,
        })
    return in_maps


def kernel(**inputs):
    nc = _CACHE.get("nc")
    if nc is None:
        nc = _CACHE["nc"] = _build()
    in_maps = _shard(inputs)
    res = run_bass_kernel_spmd(nc, in_maps, list(range(NCORES)),
                               **_CACHE.get("run_kwargs", {}))
    _CACHE["last_result"] = res
    ok = np.concatenate([res.results[m]["out_key"] for m in range(NCORES)], 0)
    ov = np.concatenate([res.results[m]["out_val"] for m in range(NCORES)], 0)
    return ok[:, None, :], ov[:, None, :]


# revision 25
# speedup vs baseline: 1.2993x; 1.0146x over previous
"""DND retrieval (episodic memory read) kernel for 8 Trainium2 NeuronCores.

Strategy: data-parallel over batch B=64 -> 8 envs per core. Per core:
  - q-side MLP chain: fp32 weights as the MOVING operand with the tiny
    [feat,8] activations stationary (avoids the very expensive fp32
    stationary-weight loads); natural-layout outputs are re-transposed
    between layers on the PE (cheap [8,128] blocks), biases added
    per-partition after the transpose. The wide Wq layer runs in bf16.
  - keys are cast f32->bf16 on DVE and transposed by the DMA xbar
    (dma_start_transpose) straight into the [k, l] layout - no PE time.
  - scores + value matmuls in bf16 with fp32 PSUM accumulation; all 8
    envs' scores accumulate into one [64, 512] PSUM bank via a
    zero-padded stationary operand.
  - rpe modulation folded into post-matmul scaling (it factors out of
    the k-contraction); validity mask built on-chip from iota + step.
  - softmax batched on a [64 (b*h), 1024 (l)] fp32 tile.
  - value_aggregator + read_memory chains: fp32 weights moving.
MLP weights are replicated per core and streamed from HBM.
"""
from contextlib import ExitStack

import numpy as np

import concourse.bass as bass
import concourse.tile as tile
from concourse import bacc, mybir
from concourse.bass_utils import run_bass_kernel_spmd
from concourse.masks import make_identity

F32 = mybir.dt.float32
BF16 = mybir.dt.bfloat16
AF = mybir.ActivationFunctionType
OP = mybir.AluOpType

L = 1024      # episode length (memory slots)
B = 64        # total batch
BL = 8        # batch per core
KD = 512      # key size
VD = 512      # value size
H = 8         # heads
MEMB = 256    # memory state embedding
SDIM = 512    # state dim
HID = 512
RIMQ = 512
LAT = KD - MEMB
NCORES = 8
LC = L // 128         # 8 l-chunks
KC = KD // 128        # 4 k-chunks
RSQK = 1.0 / np.sqrt(np.float32(KD))

_CACHE: dict = {}


def _emit(nc: bass.Bass, tc: tile.TileContext, ctx: ExitStack, io: dict):
    pool = ctx.enter_context(tc.tile_pool(name="main", bufs=1))
    kpool = ctx.enter_context(tc.tile_pool(name="keys", bufs=2))
    kbpool = ctx.enter_context(tc.tile_pool(name="keysb", bufs=2))
    vpool = ctx.enter_context(tc.tile_pool(name="vals", bufs=3))
    vbpool = ctx.enter_context(tc.tile_pool(name="valsb", bufs=4))
    wpool = ctx.enter_context(tc.tile_pool(name="wstream", bufs=2))
    wbpool = ctx.enter_context(tc.tile_pool(name="wcast", bufs=4))
    wrpool = ctx.enter_context(tc.tile_pool(name="wres", bufs=16))
    psum = ctx.enter_context(tc.tile_pool(name="ps", bufs=4, space="PSUM"))
    spsum = ctx.enter_context(tc.tile_pool(name="ps2", bufs=3, space="PSUM"))

    ident = pool.tile([128, 128], F32)
    make_identity(nc, ident[:])
    identb = pool.tile([128, 128], BF16)
    make_identity(nc, identb[:])

    def bias_tile(name, nch):
        t = pool.tile([128, nch], F32, tag="b" + name)
        nc.sync.dma_start(t[:], io[name][:])
        return t

    # natural [8, N] psum -> bf16 sbuf -> per-128-block bf16 transpose ->
    # [128, 8] bf16 tiles with per-partition bias added
    def nat_to_T(nat_psum, n, b_tile, tag):
        natsb = pool.tile([BL, n], BF16, tag=f"nat{tag}")
        nc.scalar.copy(natsb[:], nat_psum[:])
        outs = []
        for j in range(n // 128):
            tp = psum.tile([128, BL], BF16, tag="sm")
            nc.tensor.transpose(tp[:], natsb[:, j * 128:(j + 1) * 128],
                                identb[0:BL, 0:BL])
            t = pool.tile([128, BL], BF16, tag=f"{tag}{j}")
            nc.vector.tensor_scalar(out=t[:], in0=tp[:],
                                    scalar1=b_tile[:, j:j + 1],
                                    scalar2=None, op0=OP.add)
            outs.append(t)
        return outs

    # bf16 layer: activations stationary [128,8] bf16 chunks, weights
    # streamed f32 in one DMA, cast to bf16 on DVE, used as moving operand
    def layer_bf16(xT_chunks, w_name, n_out, eng=None):
        nk = len(xT_chunks)
        w = wpool.tile([128, nk, n_out], F32, tag="Wstg")
        (eng or nc.sync).dma_start(
            w[:], io[w_name].rearrange("(f p) c -> p f c", p=128))
        wb = wbpool.tile([128, nk, n_out], BF16, tag="Wstgb")
        nc.vector.tensor_copy(wb[:], w[:])
        ps = spsum.tile([BL, n_out], F32, tag="sp")
        for k in range(nk):
            nc.tensor.matmul(ps[:], xT_chunks[k][:], wb[:, k, :],
                             start=(k == 0), stop=(k == nk - 1),
                             skip_group_check=True)
        return ps

    # bf16 layer with weights STATIONARY: outputs land directly as
    # transposed [128, 8] chunks (with per-partition bias), no copies or
    # transposes between layers.
    def layer_T(xT_chunks, w_name, b_tile, n_out, tag, eng=None):
        nk = len(xT_chunks)
        w = wpool.tile([128, nk, n_out], F32, tag="Wstg")
        (eng or nc.sync).dma_start(
            w[:], io[w_name].rearrange("(f p) c -> p f c", p=128))
        wb = wbpool.tile([128, nk, n_out], BF16, tag="Wstgb")
        nc.vector.tensor_copy(wb[:], w[:])
        outs = []
        for j in range(n_out // 128):
            ps = psum.tile([128, BL], F32, tag="sm")
            for k in range(nk):
                nc.tensor.matmul(ps[:], wb[:, k, j * 128:(j + 1) * 128],
                                 xT_chunks[k][:], start=(k == 0),
                                 stop=(k == nk - 1), skip_group_check=True)
            t = pool.tile([128, BL], BF16, tag=f"{tag}{j}")
            nc.vector.tensor_scalar(out=t[:], in0=ps[:],
                                    scalar1=b_tile[:, j:j + 1],
                                    scalar2=None, op0=OP.add)
            outs.append(t)
        return outs

    # ---------------- Phase A: q-side MLP ---------------------------------
    state_nat = pool.tile([BL, SDIM], F32)
    nc.sync.dma_start(state_nat[:], io["state"][:])
    lat_nat = pool.tile([BL, LAT], F32)
    nc.sync.dma_start(lat_nat[:], io["lat"][:])

    bst = bias_tile("b_state", 2)
    bcq1 = bias_tile("bcq1", 4)
    bcq2 = bias_tile("bcq2", 4)
    bq = bias_tile("bq", 32)

    def transp_in(src_ap, n_free_chunks, tag):
        outs = []
        for c in range(n_free_chunks):
            tp = psum.tile([128, BL], F32, tag="sm")
            nc.tensor.transpose(tp[:], src_ap[:, c * 128:(c + 1) * 128],
                                ident[0:BL, 0:BL])
            t = pool.tile([128, BL], BF16, tag=tag + str(c))
            nc.vector.tensor_copy(t[:], tp[:])
            outs.append(t)
        return outs

    stateT = transp_in(state_nat, SDIM // 128, "stT")   # 4 tiles
    latT = transp_in(lat_nat, LAT // 128, "laT")        # 2 tiles

    xT = layer_T(stateT, "W_state", bst, MEMB, "xT") + latT
    h1T = layer_T(xT, "Wcq1", bcq1, HID, "h1", eng=nc.scalar)
    qcT = layer_T(h1T, "Wcq2", bcq2, KD, "qc")

    # q = qc @ Wq (bf16, weights moving), scattered into zero-padded Qpad:
    # for (b, kc) the scores lhsT is Qpad[:, kc*512 + b*64 : +64] with the
    # (b', h) columns nonzero only at b'==b, so all 8 envs' scores matmuls
    # can accumulate into one [64, 512] PSUM bank.  Bias bq added after the
    # transpose (it is per q-column = per-partition there).
    Qpad = pool.tile([128, KC * BL * B], BF16)
    nc.gpsimd.memset(Qpad[:], 0.0)
    for jg in range(4):
        wts = []
        for k in range(KC):
            w = wpool.tile([128, 1024], F32, tag="Wq")
            eng = nc.sync if k % 2 == 0 else nc.scalar
            eng.dma_start(w[:], io["Wq"][k * 128:(k + 1) * 128,
                                         jg * 1024:(jg + 1) * 1024])
            wb = wbpool.tile([128, 1024], BF16, tag="Wqb")
            nc.vector.tensor_copy(wb[:], w[:])
            wts.append(wb)
        for jj in range(8):
            j = jg * 8 + jj
            h = j // KC
            kc = j % KC
            ps = psum.tile([128, BL], F32, tag="sm")
            for k in range(KC):
                nc.tensor.matmul(ps[:], wts[k][:, jj * 128:(jj + 1) * 128],
                                 qcT[k][:], start=(k == 0),
                                 stop=(k == KC - 1), skip_group_check=True)
            base = kc * 512 + h
            nc.vector.tensor_scalar(
                out=Qpad[:, base:base + (BL - 1) * 72 + 1:72],
                in0=ps[:], scalar1=bq[:, j:j + 1],
                scalar2=None, op0=OP.add)

    # -------- Wagg: stream early, cast to bf16 on idle GpSimd, residents --
    waggb = []
    for g in range(16):
        wstg = wpool.tile([128, 2, VD], F32, tag="Waggstg")
        eng = nc.sync if g % 2 == 0 else nc.scalar
        eng.dma_start(wstg[:], io["Wagg"][g * 256:(g + 1) * 256, :]
                      .rearrange("(f p) c -> p f c", p=128))
        wgb = wrpool.tile([128, 2, VD], BF16, tag="Waggb")
        nc.gpsimd.tensor_copy(wgb[:], wstg[:])
        waggb.append(wgb)

    # ---------------- Phase B: keys (pre-transposed [K, B, L]) + scores ----
    # Keys arrive in [k, b, l] layout (relayout chosen at shard time), so
    # each [128, 4, 1024] f32 DMA slice is cast to bf16 and fed straight to
    # the PE as the moving operand.  Zero-padded lhsT -> every matmul
    # writes the full [64, 512] bank; one accumulation group per lh bank
    # spans all (kc, b).
    S = pool.tile([B, L], F32)
    sp_half0 = spsum.tile([B, 512], F32, tag="sp")
    sp_half1 = spsum.tile([B, 512], F32, tag="sp")
    sp_halves = [sp_half0, sp_half1]
    for kc in range(KC):
        for half in range(2):
            i = kc * 2 + half
            b0 = half * 4
            ktn = kpool.tile([128, 4, L], F32, tag="ktn")
            eng = nc.sync if i % 2 == 0 else nc.scalar
            eng.dma_start(ktn[:],
                          io["keysT"][kc * 128:(kc + 1) * 128, b0:b0 + 4, :])
            ktb = kbpool.tile([128, 4, L], BF16, tag="ktb")
            if i % 2 == 0:
                nc.vector.tensor_copy(ktb[:], ktn[:])
            else:
                nc.scalar.copy(ktb[:], ktn[:])
            for bl in range(4):
                b = b0 + bl
                for lh in range(2):
                    nc.tensor.matmul(sp_halves[lh][:],
                                     Qpad[:, kc * 512 + b * 64:
                                          kc * 512 + (b + 1) * 64],
                                     ktb[:, bl, lh * 512:(lh + 1) * 512],
                                     start=(kc == 0 and half == 0
                                            and bl == 0),
                                     stop=(kc == KC - 1 and half == 1
                                           and bl == 3),
                                     skip_group_check=True)
    for lh in range(2):
        nc.vector.tensor_copy(S[:, lh * 512:(lh + 1) * 512], sp_halves[lh][:])

    # ---------------- Phase C: mask + softmax ------------------------------
    iot = pool.tile([B, L], F32)
    nc.gpsimd.iota(iot[:], pattern=[[1, L]], base=0, channel_multiplier=0,
                   allow_small_or_imprecise_dtypes=True)
    stept = pool.tile([B, 1], F32)
    nc.sync.dma_start(stept[:], io["step_rep"][:])
    valid = pool.tile([B, L], F32)
    nc.vector.tensor_scalar(out=valid[:], in0=iot[:], scalar1=stept[:, 0:1],
                            scalar2=None, op0=OP.is_lt)
    A = pool.tile([B, L], F32, tag="iot")
    nc.scalar.activation(A[:], valid[:], AF.Copy, bias=-1e30, scale=1e30)

    rpeT = pool.tile([BL, L], F32)
    for lc in range(LC):
        rp = pool.tile([128, BL], F32, tag="rp")
        nc.sync.dma_start(rp[:], io["rpe"][lc * 128:(lc + 1) * 128, :])
        tp = psum.tile([BL, 128], F32, tag="sm")
        nc.tensor.transpose(tp[:], rp[:], ident[:])
        nc.vector.tensor_copy(rpeT[:, lc * 128:(lc + 1) * 128], tp[:])
    selt = pool.tile([BL, B], F32)
    nc.sync.dma_start(selt[:], io["sel"][:])
    G = pool.tile([B, L], F32)
    for lh in range(2):
        gp = spsum.tile([B, 512], F32, tag="sp")
        nc.tensor.matmul(gp[:], selt[:], rpeT[:, lh * 512:(lh + 1) * 512],
                         start=True, stop=True)
        nc.vector.tensor_tensor(out=G[:, lh * 512:(lh + 1) * 512], in0=gp[:],
                                in1=valid[:, lh * 512:(lh + 1) * 512],
                                op=OP.mult)

    nc.vector.tensor_tensor(out=S[:], in0=S[:], in1=G[:], op=OP.mult)
    nc.vector.tensor_tensor(out=S[:], in0=S[:], in1=A[:], op=OP.add)
    negM = pool.tile([B, 1], F32)
    nc.vector.tensor_reduce(out=negM[:], in_=S[:], op=OP.max,
                            axis=mybir.AxisListType.X, negate=True)
    E = pool.tile([B, L], F32, tag="G")
    Z = pool.tile([B, 1], F32)
    nc.scalar.activation(E[:], S[:], AF.Exp, bias=negM[:, 0:1], scale=1.0,
                         accum_out=Z[:, 0:1])
    R = pool.tile([B, 1], F32)
    nc.vector.reciprocal(R[:], Z[:])
    P = pool.tile([B, L], BF16, tag="rpeT")
    nc.vector.tensor_scalar(out=P[:], in0=E[:], scalar1=R[:, 0:1],
                            scalar2=None, op0=OP.mult)

    # ---------------- Phase D: prob transpose + value matmul ---------------
    PTs = []
    for lc in range(LC):
        PT = pool.tile([128, B], BF16, tag=f"PT{lc}")
        tpp = psum.tile([128, B], BF16, tag="sm")
        nc.tensor.transpose(tpp[:], P[:, lc * 128:(lc + 1) * 128],
                            identb[0:B, 0:B])
        nc.vector.tensor_copy(PT[:], tpp[:])
        PTs.append(PT)

    T = pool.tile([128, VD // 128, H, BL], BF16)
    for b in range(BL):
        rps = spsum.tile([BL, VD], F32, tag="sp")
        for lq in range(LC // 2):
            i = b * (LC // 2) + lq
            vn = vpool.tile([128, 2, VD], F32, tag="vnat")
            veng = nc.sync if i % 2 == 0 else nc.scalar
            veng.dma_start(
                vn[:], io["vals"][lq * 256:(lq + 1) * 256, b, :]
                .rearrange("(f p) c -> p f c", p=128))
            vb = vbpool.tile([128, 2, VD], BF16, tag="vb")
            if i % 2 == 0:
                nc.vector.tensor_copy(vb[:], vn[:])
            else:
                nc.scalar.copy(vb[:], vn[:])
            for f in range(2):
                lc = lq * 2 + f
                nc.tensor.matmul(rps[:], PTs[lc][:, b * H:(b + 1) * H],
                                 vb[:, f, :],
                                 start=(lc == 0), stop=(lc == LC - 1),
                                 skip_group_check=True)
        rs = pool.tile([BL, VD], BF16, tag="rs")
        nc.scalar.copy(rs[:], rps[:])
        for vs in range(VD // 128):
            tr = psum.tile([128, BL], BF16, tag="sm")
            nc.tensor.transpose(tr[:], rs[:, vs * 128:(vs + 1) * 128],
                                identb[0:BL, 0:BL])
            nc.vector.tensor_copy(T[:, vs, :, b], tr[:])

    # ---------------- Phase E: output MLP chain ----------------------------
    bagg = bias_tile("bagg", 4)
    brk1 = bias_tile("brk1", 4)
    brv1 = bias_tile("brv1", 4)

    aggp = spsum.tile([BL, VD], F32, tag="sp")
    for c in range(32):
        g, f = c // 4, c % 4
        h = c // (VD // 128)
        vs = c % (VD // 128)
        nc.tensor.matmul(aggp[:], T[:, vs, h, :], waggb[c // 2][:, c % 2, :],
                         start=(c == 0), stop=(c == 31),
                         skip_group_check=True)
    AT = nat_to_T(aggp, VD, bagg, "AT")

    # final-layer biases broadcast to [8, 512] via K=1 matmul
    ones = pool.tile([1, BL], F32)
    nc.gpsimd.memset(ones[:], 1.0)

    def bias_bcast(name):
        brow = pool.tile([1, 512], F32, tag="br" + name)
        nc.sync.dma_start(brow[:], io[name][:])
        bb = psum.tile([BL, 512], F32, tag="sm")
        nc.tensor.matmul(bb[:], ones[:], brow[:], start=True, stop=True)
        bsb = pool.tile([BL, 512], F32, tag="bs" + name)
        nc.vector.tensor_copy(bsb[:], bb[:])
        return bsb

    bk2 = bias_bcast("brk2_flat")
    bv2 = bias_bcast("brv2_flat")

    hkT = layer_T(AT, "Wrk1", brk1, HID, "hk")
    ok_ps = layer_bf16(hkT, "Wrk2", RIMQ)
    hvT = layer_T(AT, "Wrv1", brv1, HID, "hv", eng=nc.scalar)
    ov_ps = layer_bf16(hvT, "Wrv2", VD, eng=nc.scalar)

    for name, ps_, bias_sb in (("out_key", ok_ps, bk2), ("out_val", ov_ps, bv2)):
        onat = pool.tile([BL, 512], F32, tag="o" + name)
        nc.vector.tensor_tensor(out=onat[:], in0=ps_[:], in1=bias_sb[:],
                                op=OP.add)
        nc.sync.dma_start(io[name][:], onat[:])


def _build():
    nc = bacc.Bacc("TRN2", target_bir_lowering=False, debug=False,
                   num_devices=NCORES)
    io = {}

    def din(name, shape):
        io[name] = nc.dram_tensor(name, shape, F32, kind="ExternalInput").ap()

    din("keysT", [KD, BL, L])
    din("vals", [L, BL, VD])
    din("rpe", [L, BL])
    din("step_rep", [B, 1])
    din("state", [BL, SDIM])
    din("lat", [BL, LAT])
    din("sel", [BL, B])
    din("W_state", [SDIM, MEMB])
    din("b_state", [128, 2])
    din("Wcq1", [KD, HID])
    din("bcq1", [128, 4])
    din("Wcq2", [HID, KD])
    din("bcq2", [128, 4])
    din("Wq", [KD, H * KD])
    din("bq", [128, 32])
    din("Wagg", [H * VD, VD])
    din("bagg", [128, 4])
    din("Wrk1", [VD, HID])
    din("brk1", [128, 4])
    din("Wrk2", [HID, RIMQ])
    din("brk2_flat", [1, 512])
    din("Wrv1", [VD, HID])
    din("brv1", [128, 4])
    din("Wrv2", [HID, VD])
    din("brv2_flat", [1, 512])
    io["out_key"] = nc.dram_tensor("out_key", [BL, RIMQ], F32,
                                   kind="ExternalOutput").ap()
    io["out_val"] = nc.dram_tensor("out_val", [BL, VD], F32,
                                   kind="ExternalOutput").ap()

    with tile.TileContext(nc) as tc, ExitStack() as ctx:
        _emit(nc, tc, ctx, io)
    nc.compile()
    return nc


def _rsb(bias, nch):
    return np.ascontiguousarray(
        np.asarray(bias, np.float32).reshape(nch, 128).T)


def _shard(inputs):
    f = lambda x: np.asarray(x, np.float32)
    keys, vals, rpe = f(inputs["keys"]), f(inputs["vals"]), f(inputs["rpe_mod"])
    step = np.asarray(inputs["step"]).astype(np.float32)
    state, lat = f(inputs["state"]), f(inputs["task_inference_latent"])
    sel = np.ascontiguousarray(
        np.repeat(np.eye(BL, dtype=np.float32), BL, axis=1) * RSQK)
    shared = {
        "sel": sel,
        "W_state": f(inputs["W_state"]), "b_state": _rsb(inputs["b_state"], 2),
        "Wcq1": f(inputs["Wcq1"]), "bcq1": _rsb(inputs["bcq1"], 4),
        "Wcq2": f(inputs["Wcq2"]), "bcq2": _rsb(inputs["bcq2"], 4),
        "Wq": f(inputs["Wq"]), "bq": _rsb(inputs["bq"], 32),
        "Wagg": f(inputs["Wagg"]), "bagg": _rsb(inputs["bagg"], 4),
        "Wrk1": f(inputs["Wrk1"]), "brk1": _rsb(inputs["brk1"], 4),
        "Wrk2": f(inputs["Wrk2"]),
        "brk2_flat": np.ascontiguousarray(f(inputs["brk2"])[None, :]),
        "Wrv1": f(inputs["Wrv1"]), "brv1": _rsb(inputs["brv1"], 4),
        "Wrv2": f(inputs["Wrv2"]),
        "brv2_flat": np.ascontiguousarray(f(inputs["brv2"])[None, :]),
    }
    in_maps = []
    for m in range(NCORES):
        b0 = m * BL
        in_maps.append({
            "keysT": np.ascontiguousarray(
                keys[:, b0:b0 + BL, :].transpose(2, 1, 0)),
            "vals": np.ascontiguousarray(vals[:, b0:b0 + BL, :]),
            "rpe": np.ascontiguousarray(rpe[:, b0:b0 + BL, 0]),
            "step_rep": np.ascontiguousarray(
                np.repeat(step[b0:b0 + BL], H)[:, None]),
            "state": np.ascontiguousarray(state[b0:b0 + BL]),
            "lat": np.ascontiguousarray(lat[b0:b0 + BL]),
            **shared,
        })
    return in_maps


def kernel(**inputs):
    nc = _CACHE.get("nc")
    if nc is None:
        nc = _CACHE["nc"] = _build()
    in_maps = _shard(inputs)
    res = run_bass_kernel_spmd(nc, in_maps, list(range(NCORES)),
                               **_CACHE.get("run_kwargs", {}))
    _CACHE["last_result"] = res
    ok = np.concatenate([res.results[m]["out_key"] for m in range(NCORES)], 0)
    ov = np.concatenate([res.results[m]["out_val"] for m in range(NCORES)], 0)
    return ok[:, None, :], ov[:, None, :]


# revision 26
# speedup vs baseline: 1.3415x; 1.0325x over previous
"""DND retrieval (episodic memory read) kernel for 8 Trainium2 NeuronCores.

Strategy: data-parallel over batch B=64 -> 8 envs per core. Per core:
  - q-side MLP chain: fp32 weights as the MOVING operand with the tiny
    [feat,8] activations stationary (avoids the very expensive fp32
    stationary-weight loads); natural-layout outputs are re-transposed
    between layers on the PE (cheap [8,128] blocks), biases added
    per-partition after the transpose. The wide Wq layer runs in bf16.
  - keys are cast f32->bf16 on DVE and transposed by the DMA xbar
    (dma_start_transpose) straight into the [k, l] layout - no PE time.
  - scores + value matmuls in bf16 with fp32 PSUM accumulation; all 8
    envs' scores accumulate into one [64, 512] PSUM bank via a
    zero-padded stationary operand.
  - rpe modulation folded into post-matmul scaling (it factors out of
    the k-contraction); validity mask built on-chip from iota + step.
  - softmax batched on a [64 (b*h), 1024 (l)] fp32 tile.
  - value_aggregator + read_memory chains: fp32 weights moving.
MLP weights are replicated per core and streamed from HBM.
"""
from contextlib import ExitStack

import numpy as np

import concourse.bass as bass
import concourse.tile as tile
from concourse import bacc, mybir
from concourse.bass_utils import run_bass_kernel_spmd
from concourse.masks import make_identity

F32 = mybir.dt.float32
BF16 = mybir.dt.bfloat16
AF = mybir.ActivationFunctionType
OP = mybir.AluOpType

L = 1024      # episode length (memory slots)
B = 64        # total batch
BL = 8        # batch per core
KD = 512      # key size
VD = 512      # value size
H = 8         # heads
MEMB = 256    # memory state embedding
SDIM = 512    # state dim
HID = 512
RIMQ = 512
LAT = KD - MEMB
NCORES = 8
LC = L // 128         # 8 l-chunks
KC = KD // 128        # 4 k-chunks
RSQK = 1.0 / np.sqrt(np.float32(KD))

_CACHE: dict = {}


def _emit(nc: bass.Bass, tc: tile.TileContext, ctx: ExitStack, io: dict):
    pool = ctx.enter_context(tc.tile_pool(name="main", bufs=1))
    kpool = ctx.enter_context(tc.tile_pool(name="keys", bufs=3))
    kbpool = ctx.enter_context(tc.tile_pool(name="keysb", bufs=3))
    vpool = ctx.enter_context(tc.tile_pool(name="vals", bufs=3))
    vbpool = ctx.enter_context(tc.tile_pool(name="valsb", bufs=4))
    wpool = ctx.enter_context(tc.tile_pool(name="wstream", bufs=2))
    wbpool = ctx.enter_context(tc.tile_pool(name="wcast", bufs=4))
    wrpool = ctx.enter_context(tc.tile_pool(name="wres", bufs=16))
    psum = ctx.enter_context(tc.tile_pool(name="ps", bufs=4, space="PSUM"))
    spsum = ctx.enter_context(tc.tile_pool(name="ps2", bufs=3, space="PSUM"))

    ident = pool.tile([128, 128], F32)
    make_identity(nc, ident[:])
    identb = pool.tile([128, 128], BF16)
    make_identity(nc, identb[:])

    def bias_tile(name, nch):
        t = pool.tile([128, nch], F32, tag="b" + name)
        nc.sync.dma_start(t[:], io[name][:])
        return t

    # natural [8, N] psum -> bf16 sbuf -> per-128-block bf16 transpose ->
    # [128, 8] bf16 tiles with per-partition bias added
    def nat_to_T(nat_psum, n, b_tile, tag):
        natsb = pool.tile([BL, n], BF16, tag=f"nat{tag}")
        nc.scalar.copy(natsb[:], nat_psum[:])
        outs = []
        for j in range(n // 128):
            tp = psum.tile([128, BL], BF16, tag="sm")
            nc.tensor.transpose(tp[:], natsb[:, j * 128:(j + 1) * 128],
                                identb[0:BL, 0:BL])
            t = pool.tile([128, BL], BF16, tag=f"{tag}{j}")
            nc.vector.tensor_scalar(out=t[:], in0=tp[:],
                                    scalar1=b_tile[:, j:j + 1],
                                    scalar2=None, op0=OP.add)
            outs.append(t)
        return outs

    # bf16 layer: activations stationary [128,8] bf16 chunks, weights
    # streamed f32 in one DMA, cast to bf16 on DVE, used as moving operand
    def layer_bf16(xT_chunks, w_name, n_out, eng=None):
        nk = len(xT_chunks)
        w = wpool.tile([128, nk, n_out], F32, tag="Wstg")
        (eng or nc.sync).dma_start(
            w[:], io[w_name].rearrange("(f p) c -> p f c", p=128))
        wb = wbpool.tile([128, nk, n_out], BF16, tag="Wstgb")
        nc.vector.tensor_copy(wb[:], w[:])
        ps = spsum.tile([BL, n_out], F32, tag="sp")
        for k in range(nk):
            nc.tensor.matmul(ps[:], xT_chunks[k][:], wb[:, k, :],
                             start=(k == 0), stop=(k == nk - 1),
                             skip_group_check=True)
        return ps

    # bf16 layer with weights STATIONARY: outputs land directly as
    # transposed [128, 8] chunks (with per-partition bias), no copies or
    # transposes between layers.
    def layer_T(xT_chunks, w_name, b_tile, n_out, tag, eng=None):
        nk = len(xT_chunks)
        w = wpool.tile([128, nk, n_out], F32, tag="Wstg")
        (eng or nc.sync).dma_start(
            w[:], io[w_name].rearrange("(f p) c -> p f c", p=128))
        wb = wbpool.tile([128, nk, n_out], BF16, tag="Wstgb")
        nc.vector.tensor_copy(wb[:], w[:])
        outs = []
        for j in range(n_out // 128):
            ps = psum.tile([128, BL], F32, tag="sm")
            for k in range(nk):
                nc.tensor.matmul(ps[:], wb[:, k, j * 128:(j + 1) * 128],
                                 xT_chunks[k][:], start=(k == 0),
                                 stop=(k == nk - 1), skip_group_check=True)
            t = pool.tile([128, BL], BF16, tag=f"{tag}{j}")
            nc.vector.tensor_scalar(out=t[:], in0=ps[:],
                                    scalar1=b_tile[:, j:j + 1],
                                    scalar2=None, op0=OP.add)
            outs.append(t)
        return outs

    # ---------------- Phase A: q-side MLP ---------------------------------
    state_nat = pool.tile([BL, SDIM], F32)
    nc.sync.dma_start(state_nat[:], io["state"][:])
    lat_nat = pool.tile([BL, LAT], F32)
    nc.sync.dma_start(lat_nat[:], io["lat"][:])

    bst = bias_tile("b_state", 2)
    bcq1 = bias_tile("bcq1", 4)
    bcq2 = bias_tile("bcq2", 4)
    bq = bias_tile("bq", 32)

    def transp_in(src_ap, n_free_chunks, tag):
        outs = []
        for c in range(n_free_chunks):
            tp = psum.tile([128, BL], F32, tag="sm")
            nc.tensor.transpose(tp[:], src_ap[:, c * 128:(c + 1) * 128],
                                ident[0:BL, 0:BL])
            t = pool.tile([128, BL], BF16, tag=tag + str(c))
            nc.vector.tensor_copy(t[:], tp[:])
            outs.append(t)
        return outs

    stateT = transp_in(state_nat, SDIM // 128, "stT")   # 4 tiles
    latT = transp_in(lat_nat, LAT // 128, "laT")        # 2 tiles

    xT = layer_T(stateT, "W_state", bst, MEMB, "xT") + latT
    h1T = layer_T(xT, "Wcq1", bcq1, HID, "h1", eng=nc.scalar)
    qcT = layer_T(h1T, "Wcq2", bcq2, KD, "qc")

    # q = qc @ Wq (bf16, weights moving), scattered into zero-padded Qpad:
    # for (b, kc) the scores lhsT is Qpad[:, kc*512 + b*64 : +64] with the
    # (b', h) columns nonzero only at b'==b, so all 8 envs' scores matmuls
    # can accumulate into one [64, 512] PSUM bank.  Bias bq added after the
    # transpose (it is per q-column = per-partition there).
    Qpad = pool.tile([128, KC * BL * B], BF16)
    nc.gpsimd.memset(Qpad[:], 0.0)
    for jg in range(4):
        wts = []
        for k in range(KC):
            w = wpool.tile([128, 1024], F32, tag="Wq")
            eng = nc.sync if k % 2 == 0 else nc.scalar
            eng.dma_start(w[:], io["Wq"][k * 128:(k + 1) * 128,
                                         jg * 1024:(jg + 1) * 1024])
            wb = wbpool.tile([128, 1024], BF16, tag="Wqb")
            nc.vector.tensor_copy(wb[:], w[:])
            wts.append(wb)
        for jj in range(8):
            j = jg * 8 + jj
            h = j // KC
            kc = j % KC
            ps = psum.tile([128, BL], F32, tag="sm")
            for k in range(KC):
                nc.tensor.matmul(ps[:], wts[k][:, jj * 128:(jj + 1) * 128],
                                 qcT[k][:], start=(k == 0),
                                 stop=(k == KC - 1), skip_group_check=True)
            base = kc * 512 + h
            nc.vector.tensor_scalar(
                out=Qpad[:, base:base + (BL - 1) * 72 + 1:72],
                in0=ps[:], scalar1=bq[:, j:j + 1],
                scalar2=None, op0=OP.add)

    # -------- Wagg: stream early, cast to bf16 on idle GpSimd, residents --
    waggb = []
    for g in range(16):
        wstg = wpool.tile([128, 2, VD], F32, tag="Waggstg")
        engs2 = [nc.sync, nc.scalar, nc.gpsimd]
        engs2[g % 3].dma_start(wstg[:], io["Wagg"][g * 256:(g + 1) * 256, :]
                      .rearrange("(f p) c -> p f c", p=128))
        wgb = wrpool.tile([128, 2, VD], BF16, tag="Waggb")
        nc.gpsimd.tensor_copy(wgb[:], wstg[:])
        waggb.append(wgb)

    # ---------------- Phase B: keys (pre-transposed [K, B, L]) + scores ----
    # Keys arrive in [k, b, l] layout (relayout chosen at shard time), so
    # each [128, 4, 1024] f32 DMA slice is cast to bf16 and fed straight to
    # the PE as the moving operand.  Zero-padded lhsT -> every matmul
    # writes the full [64, 512] bank; one accumulation group per lh bank
    # spans all (kc, b).
    S = pool.tile([B, L], F32)
    sp_half0 = spsum.tile([B, 512], F32, tag="sp")
    sp_half1 = spsum.tile([B, 512], F32, tag="sp")
    sp_halves = [sp_half0, sp_half1]
    engs = [nc.sync, nc.scalar, nc.gpsimd]
    for kc in range(KC):
        for q in range(4):
            i = kc * 4 + q
            b0 = q * 2
            ktn = kpool.tile([128, 2, L], F32, tag="ktn")
            engs[i % 3].dma_start(
                ktn[:], io["keysT"][kc * 128:(kc + 1) * 128, b0:b0 + 2, :])
            ktb = kbpool.tile([128, 2, L], BF16, tag="ktb")
            if i % 2 == 0:
                nc.vector.tensor_copy(ktb[:], ktn[:])
            else:
                nc.scalar.copy(ktb[:], ktn[:])
            for bl in range(2):
                b = b0 + bl
                for lh in range(2):
                    nc.tensor.matmul(sp_halves[lh][:],
                                     Qpad[:, kc * 512 + b * 64:
                                          kc * 512 + (b + 1) * 64],
                                     ktb[:, bl, lh * 512:(lh + 1) * 512],
                                     start=(kc == 0 and q == 0 and bl == 0),
                                     stop=(kc == KC - 1 and q == 3
                                           and bl == 1),
                                     skip_group_check=True)
    for lh in range(2):
        nc.vector.tensor_copy(S[:, lh * 512:(lh + 1) * 512], sp_halves[lh][:])

    # ---------------- Phase C: mask + softmax ------------------------------
    iot = pool.tile([B, L], F32)
    nc.gpsimd.iota(iot[:], pattern=[[1, L]], base=0, channel_multiplier=0,
                   allow_small_or_imprecise_dtypes=True)
    stept = pool.tile([B, 1], F32)
    nc.sync.dma_start(stept[:], io["step_rep"][:])
    valid = pool.tile([B, L], F32)
    nc.vector.tensor_scalar(out=valid[:], in0=iot[:], scalar1=stept[:, 0:1],
                            scalar2=None, op0=OP.is_lt)
    A = pool.tile([B, L], F32, tag="iot")
    nc.scalar.activation(A[:], valid[:], AF.Copy, bias=-1e30, scale=1e30)

    rpeT = pool.tile([BL, L], F32)
    for lc in range(LC):
        rp = pool.tile([128, BL], F32, tag="rp")
        nc.sync.dma_start(rp[:], io["rpe"][lc * 128:(lc + 1) * 128, :])
        tp = psum.tile([BL, 128], F32, tag="sm")
        nc.tensor.transpose(tp[:], rp[:], ident[:])
        nc.vector.tensor_copy(rpeT[:, lc * 128:(lc + 1) * 128], tp[:])
    selt = pool.tile([BL, B], F32)
    nc.sync.dma_start(selt[:], io["sel"][:])
    G = pool.tile([B, L], F32)
    for lh in range(2):
        gp = spsum.tile([B, 512], F32, tag="sp")
        nc.tensor.matmul(gp[:], selt[:], rpeT[:, lh * 512:(lh + 1) * 512],
                         start=True, stop=True)
        nc.vector.tensor_tensor(out=G[:, lh * 512:(lh + 1) * 512], in0=gp[:],
                                in1=valid[:, lh * 512:(lh + 1) * 512],
                                op=OP.mult)

    nc.vector.tensor_tensor(out=S[:], in0=S[:], in1=G[:], op=OP.mult)
    nc.vector.tensor_tensor(out=S[:], in0=S[:], in1=A[:], op=OP.add)
    negM = pool.tile([B, 1], F32)
    nc.vector.tensor_reduce(out=negM[:], in_=S[:], op=OP.max,
                            axis=mybir.AxisListType.X, negate=True)
    E = pool.tile([B, L], F32, tag="G")
    Z = pool.tile([B, 1], F32)
    nc.scalar.activation(E[:], S[:], AF.Exp, bias=negM[:, 0:1], scale=1.0,
                         accum_out=Z[:, 0:1])
    R = pool.tile([B, 1], F32)
    nc.vector.reciprocal(R[:], Z[:])
    P = pool.tile([B, L], BF16, tag="rpeT")
    nc.vector.tensor_scalar(out=P[:], in0=E[:], scalar1=R[:, 0:1],
                            scalar2=None, op0=OP.mult)

    # ---------------- Phase D: prob transpose + value matmul ---------------
    PTs = []
    for lc in range(LC):
        PT = pool.tile([128, B], BF16, tag=f"PT{lc}")
        tpp = psum.tile([128, B], BF16, tag="sm")
        nc.tensor.transpose(tpp[:], P[:, lc * 128:(lc + 1) * 128],
                            identb[0:B, 0:B])
        nc.vector.tensor_copy(PT[:], tpp[:])
        PTs.append(PT)

    T = pool.tile([128, VD // 128, H, BL], BF16)
    for b in range(BL):
        rps = spsum.tile([BL, VD], F32, tag="sp")
        for lq in range(LC // 2):
            i = b * (LC // 2) + lq
            vn = vpool.tile([128, 2, VD], F32, tag="vnat")
            veng = engs[i % 3]
            veng.dma_start(
                vn[:], io["vals"][lq * 256:(lq + 1) * 256, b, :]
                .rearrange("(f p) c -> p f c", p=128))
            vb = vbpool.tile([128, 2, VD], BF16, tag="vb")
            if i % 2 == 0:
                nc.vector.tensor_copy(vb[:], vn[:])
            else:
                nc.scalar.copy(vb[:], vn[:])
            for f in range(2):
                lc = lq * 2 + f
                nc.tensor.matmul(rps[:], PTs[lc][:, b * H:(b + 1) * H],
                                 vb[:, f, :],
                                 start=(lc == 0), stop=(lc == LC - 1),
                                 skip_group_check=True)
        rs = pool.tile([BL, VD], BF16, tag="rs")
        nc.scalar.copy(rs[:], rps[:])
        for vs in range(VD // 128):
            tr = psum.tile([128, BL], BF16, tag="sm")
            nc.tensor.transpose(tr[:], rs[:, vs * 128:(vs + 1) * 128],
                                identb[0:BL, 0:BL])
            nc.vector.tensor_copy(T[:, vs, :, b], tr[:])

    # ---------------- Phase E: output MLP chain ----------------------------
    bagg = bias_tile("bagg", 4)
    brk1 = bias_tile("brk1", 4)
    brv1 = bias_tile("brv1", 4)

    aggp = spsum.tile([BL, VD], F32, tag="sp")
    for c in range(32):
        g, f = c // 4, c % 4
        h = c // (VD // 128)
        vs = c % (VD // 128)
        nc.tensor.matmul(aggp[:], T[:, vs, h, :], waggb[c // 2][:, c % 2, :],
                         start=(c == 0), stop=(c == 31),
                         skip_group_check=True)
    AT = nat_to_T(aggp, VD, bagg, "AT")

    # final-layer biases broadcast to [8, 512] via K=1 matmul
    ones = pool.tile([1, BL], F32)
    nc.gpsimd.memset(ones[:], 1.0)

    def bias_bcast(name):
        brow = pool.tile([1, 512], F32, tag="br" + name)
        nc.sync.dma_start(brow[:], io[name][:])
        bb = psum.tile([BL, 512], F32, tag="sm")
        nc.tensor.matmul(bb[:], ones[:], brow[:], start=True, stop=True)
        bsb = pool.tile([BL, 512], F32, tag="bs" + name)
        nc.vector.tensor_copy(bsb[:], bb[:])
        return bsb

    bk2 = bias_bcast("brk2_flat")
    bv2 = bias_bcast("brv2_flat")

    hkT = layer_T(AT, "Wrk1", brk1, HID, "hk")
    ok_ps = layer_bf16(hkT, "Wrk2", RIMQ)
    hvT = layer_T(AT, "Wrv1", brv1, HID, "hv", eng=nc.scalar)
    ov_ps = layer_bf16(hvT, "Wrv2", VD, eng=nc.scalar)

    for name, ps_, bias_sb in (("out_key", ok_ps, bk2), ("out_val", ov_ps, bv2)):
        onat = pool.tile([BL, 512], F32, tag="o" + name)
        nc.vector.tensor_tensor(out=onat[:], in0=ps_[:], in1=bias_sb[:],
                                op=OP.add)
        nc.sync.dma_start(io[name][:], onat[:])


def _build():
    nc = bacc.Bacc("TRN2", target_bir_lowering=False, debug=False,
                   num_devices=NCORES)
    io = {}

    def din(name, shape):
        io[name] = nc.dram_tensor(name, shape, F32, kind="ExternalInput").ap()

    din("keysT", [KD, BL, L])
    din("vals", [L, BL, VD])
    din("rpe", [L, BL])
    din("step_rep", [B, 1])
    din("state", [BL, SDIM])
    din("lat", [BL, LAT])
    din("sel", [BL, B])
    din("W_state", [SDIM, MEMB])
    din("b_state", [128, 2])
    din("Wcq1", [KD, HID])
    din("bcq1", [128, 4])
    din("Wcq2", [HID, KD])
    din("bcq2", [128, 4])
    din("Wq", [KD, H * KD])
    din("bq", [128, 32])
    din("Wagg", [H * VD, VD])
    din("bagg", [128, 4])
    din("Wrk1", [VD, HID])
    din("brk1", [128, 4])
    din("Wrk2", [HID, RIMQ])
    din("brk2_flat", [1, 512])
    din("Wrv1", [VD, HID])
    din("brv1", [128, 4])
    din("Wrv2", [HID, VD])
    din("brv2_flat", [1, 512])
    io["out_key"] = nc.dram_tensor("out_key", [BL, RIMQ], F32,
                                   kind="ExternalOutput").ap()
    io["out_val"] = nc.dram_tensor("out_val", [BL, VD], F32,
                                   kind="ExternalOutput").ap()

    with tile.TileContext(nc) as tc, ExitStack() as ctx:
        _emit(nc, tc, ctx, io)
    nc.compile()
    return nc


def _rsb(bias, nch):
    return np.ascontiguousarray(
        np.asarray(bias, np.float32).reshape(nch, 128).T)


def _shard(inputs):
    f = lambda x: np.asarray(x, np.float32)
    keys, vals, rpe = f(inputs["keys"]), f(inputs["vals"]), f(inputs["rpe_mod"])
    step = np.asarray(inputs["step"]).astype(np.float32)
    state, lat = f(inputs["state"]), f(inputs["task_inference_latent"])
    sel = np.ascontiguousarray(
        np.repeat(np.eye(BL, dtype=np.float32), BL, axis=1) * RSQK)
    shared = {
        "sel": sel,
        "W_state": f(inputs["W_state"]), "b_state": _rsb(inputs["b_state"], 2),
        "Wcq1": f(inputs["Wcq1"]), "bcq1": _rsb(inputs["bcq1"], 4),
        "Wcq2": f(inputs["Wcq2"]), "bcq2": _rsb(inputs["bcq2"], 4),
        "Wq": f(inputs["Wq"]), "bq": _rsb(inputs["bq"], 32),
        "Wagg": f(inputs["Wagg"]), "bagg": _rsb(inputs["bagg"], 4),
        "Wrk1": f(inputs["Wrk1"]), "brk1": _rsb(inputs["brk1"], 4),
        "Wrk2": f(inputs["Wrk2"]),
        "brk2_flat": np.ascontiguousarray(f(inputs["brk2"])[None, :]),
        "Wrv1": f(inputs["Wrv1"]), "brv1": _rsb(inputs["brv1"], 4),
        "Wrv2": f(inputs["Wrv2"]),
        "brv2_flat": np.ascontiguousarray(f(inputs["brv2"])[None, :]),
    }
    in_maps = []
    for m in range(NCORES):
        b0 = m * BL
        in_maps.append({
            "keysT": np.ascontiguousarray(
                keys[:, b0:b0 + BL, :].transpose(2, 1, 0)),
            "vals": np.ascontiguousarray(vals[:, b0:b0 + BL, :]),
            "rpe": np.ascontiguousarray(rpe[:, b0:b0 + BL, 0]),
            "step_rep": np.ascontiguousarray(
                np.repeat(step[b0:b0 + BL], H)[:, None]),
            "state": np.ascontiguousarray(state[b0:b0 + BL]),
            "lat": np.ascontiguousarray(lat[b0:b0 + BL]),
            **shared,
        })
    return in_maps


def kernel(**inputs):
    nc = _CACHE.get("nc")
    if nc is None:
        nc = _CACHE["nc"] = _build()
    in_maps = _shard(inputs)
    res = run_bass_kernel_spmd(nc, in_maps, list(range(NCORES)),
                               **_CACHE.get("run_kwargs", {}))
    _CACHE["last_result"] = res
    ok = np.concatenate([res.results[m]["out_key"] for m in range(NCORES)], 0)
    ov = np.concatenate([res.results[m]["out_val"] for m in range(NCORES)], 0)
    return ok[:, None, :], ov[:, None, :]


# revision 27
# speedup vs baseline: 1.4514x; 1.0819x over previous
"""DND retrieval (episodic memory read) kernel for 8 Trainium2 NeuronCores.

Strategy: data-parallel over batch B=64 -> 8 envs per core. Per core:
  - q-side MLP chain: fp32 weights as the MOVING operand with the tiny
    [feat,8] activations stationary (avoids the very expensive fp32
    stationary-weight loads); natural-layout outputs are re-transposed
    between layers on the PE (cheap [8,128] blocks), biases added
    per-partition after the transpose. The wide Wq layer runs in bf16.
  - keys are cast f32->bf16 on DVE and transposed by the DMA xbar
    (dma_start_transpose) straight into the [k, l] layout - no PE time.
  - scores + value matmuls in bf16 with fp32 PSUM accumulation; all 8
    envs' scores accumulate into one [64, 512] PSUM bank via a
    zero-padded stationary operand.
  - rpe modulation folded into post-matmul scaling (it factors out of
    the k-contraction); validity mask built on-chip from iota + step.
  - softmax batched on a [64 (b*h), 1024 (l)] fp32 tile.
  - value_aggregator + read_memory chains: fp32 weights moving.
MLP weights are replicated per core and streamed from HBM.
"""
from contextlib import ExitStack

import numpy as np

import concourse.bass as bass
import concourse.tile as tile
from concourse import bacc, mybir
from concourse.bass_utils import run_bass_kernel_spmd
from concourse.masks import make_identity

F32 = mybir.dt.float32
BF16 = mybir.dt.bfloat16
AF = mybir.ActivationFunctionType
OP = mybir.AluOpType

L = 1024      # episode length (memory slots)
B = 64        # total batch
BL = 8        # batch per core
KD = 512      # key size
VD = 512      # value size
H = 8         # heads
MEMB = 256    # memory state embedding
SDIM = 512    # state dim
HID = 512
RIMQ = 512
LAT = KD - MEMB
NCORES = 8
LC = L // 128         # 8 l-chunks
KC = KD // 128        # 4 k-chunks
RSQK = 1.0 / np.sqrt(np.float32(KD))

_CACHE: dict = {}


def _emit(nc: bass.Bass, tc: tile.TileContext, ctx: ExitStack, io: dict):
    pool = ctx.enter_context(tc.tile_pool(name="main", bufs=1))
    kpool = ctx.enter_context(tc.tile_pool(name="keys", bufs=3))
    kbpool = ctx.enter_context(tc.tile_pool(name="keysb", bufs=3))
    vpool = ctx.enter_context(tc.tile_pool(name="vals", bufs=4))
    vbpool = ctx.enter_context(tc.tile_pool(name="valsb", bufs=5))
    wpool = ctx.enter_context(tc.tile_pool(name="wstream", bufs=2))
    wbpool = ctx.enter_context(tc.tile_pool(name="wcast", bufs=4))
    wrpool = ctx.enter_context(tc.tile_pool(name="wres", bufs=16))
    psum = ctx.enter_context(tc.tile_pool(name="ps", bufs=5, space="PSUM"))
    spsum = ctx.enter_context(tc.tile_pool(name="ps2", bufs=3, space="PSUM"))

    ident = pool.tile([128, 128], F32)
    make_identity(nc, ident[:])
    identb = pool.tile([128, 128], BF16)
    make_identity(nc, identb[:])

    def bias_tile(name, nch):
        t = pool.tile([128, nch], F32, tag="b" + name)
        nc.sync.dma_start(t[:], io[name][:])
        return t

    # natural [8, N] psum -> bf16 sbuf -> per-128-block bf16 transpose ->
    # [128, 8] bf16 tiles with per-partition bias added
    def nat_to_T(nat_psum, n, b_tile, tag):
        natsb = pool.tile([BL, n], BF16, tag=f"nat{tag}")
        nc.scalar.copy(natsb[:], nat_psum[:])
        outs = []
        for j in range(n // 128):
            tp = psum.tile([128, BL], BF16, tag="sm")
            nc.tensor.transpose(tp[:], natsb[:, j * 128:(j + 1) * 128],
                                identb[0:BL, 0:BL])
            t = pool.tile([128, BL], BF16, tag=f"{tag}{j}")
            nc.vector.tensor_scalar(out=t[:], in0=tp[:],
                                    scalar1=b_tile[:, j:j + 1],
                                    scalar2=None, op0=OP.add)
            outs.append(t)
        return outs

    # bf16 layer: activations stationary [128,8] bf16 chunks, weights
    # streamed f32 in one DMA, cast to bf16 on DVE, used as moving operand
    def layer_bf16(xT_chunks, w_name, n_out, eng=None):
        nk = len(xT_chunks)
        w = wpool.tile([128, nk, n_out], F32, tag="Wstg")
        (eng or nc.sync).dma_start(
            w[:], io[w_name].rearrange("(f p) c -> p f c", p=128))
        wb = wbpool.tile([128, nk, n_out], BF16, tag="Wstgb")
        nc.vector.tensor_copy(wb[:], w[:])
        ps = spsum.tile([BL, n_out], F32, tag="sp")
        for k in range(nk):
            nc.tensor.matmul(ps[:], xT_chunks[k][:], wb[:, k, :],
                             start=(k == 0), stop=(k == nk - 1),
                             skip_group_check=True)
        return ps

    # bf16 layer with weights STATIONARY: outputs land directly as
    # transposed [128, 8] chunks (with per-partition bias), no copies or
    # transposes between layers.
    def layer_T(xT_chunks, w_name, b_tile, n_out, tag, eng=None):
        nk = len(xT_chunks)
        w = wpool.tile([128, nk, n_out], F32, tag="Wstg")
        (eng or nc.sync).dma_start(
            w[:], io[w_name].rearrange("(f p) c -> p f c", p=128))
        wb = wbpool.tile([128, nk, n_out], BF16, tag="Wstgb")
        nc.vector.tensor_copy(wb[:], w[:])
        outs = []
        for j in range(n_out // 128):
            ps = psum.tile([128, BL], F32, tag="sm")
            for k in range(nk):
                nc.tensor.matmul(ps[:], wb[:, k, j * 128:(j + 1) * 128],
                                 xT_chunks[k][:], start=(k == 0),
                                 stop=(k == nk - 1), skip_group_check=True)
            t = pool.tile([128, BL], BF16, tag=f"{tag}{j}")
            nc.vector.tensor_scalar(out=t[:], in0=ps[:],
                                    scalar1=b_tile[:, j:j + 1],
                                    scalar2=None, op0=OP.add)
            outs.append(t)
        return outs

    # ---------------- Phase A: q-side MLP ---------------------------------
    state_nat = pool.tile([BL, SDIM], F32)
    nc.sync.dma_start(state_nat[:], io["state"][:])
    lat_nat = pool.tile([BL, LAT], F32)
    nc.sync.dma_start(lat_nat[:], io["lat"][:])

    bst = bias_tile("b_state", 2)
    bcq1 = bias_tile("bcq1", 4)
    bcq2 = bias_tile("bcq2", 4)
    bq = bias_tile("bq", 32)

    def transp_in(src_ap, n_free_chunks, tag):
        outs = []
        for c in range(n_free_chunks):
            tp = psum.tile([128, BL], F32, tag="sm")
            nc.tensor.transpose(tp[:], src_ap[:, c * 128:(c + 1) * 128],
                                ident[0:BL, 0:BL])
            t = pool.tile([128, BL], BF16, tag=tag + str(c))
            nc.vector.tensor_copy(t[:], tp[:])
            outs.append(t)
        return outs

    stateT = transp_in(state_nat, SDIM // 128, "stT")   # 4 tiles
    latT = transp_in(lat_nat, LAT // 128, "laT")        # 2 tiles

    xT = layer_T(stateT, "W_state", bst, MEMB, "xT") + latT
    h1T = layer_T(xT, "Wcq1", bcq1, HID, "h1", eng=nc.scalar)
    qcT = layer_T(h1T, "Wcq2", bcq2, KD, "qc")

    # q = qc @ Wq (bf16, weights moving), scattered into zero-padded Qpad:
    # for (b, kc) the scores lhsT is Qpad[:, kc*512 + b*64 : +64] with the
    # (b', h) columns nonzero only at b'==b, so all 8 envs' scores matmuls
    # can accumulate into one [64, 512] PSUM bank.  Bias bq added after the
    # transpose (it is per q-column = per-partition there).
    Qpad = pool.tile([128, KC * BL * B], BF16)
    nc.gpsimd.memset(Qpad[:], 0.0)
    for jg in range(4):
        wts = []
        for k in range(KC):
            w = wpool.tile([128, 1024], F32, tag="Wq")
            eng = nc.sync if k % 2 == 0 else nc.scalar
            eng.dma_start(w[:], io["Wq"][k * 128:(k + 1) * 128,
                                         jg * 1024:(jg + 1) * 1024])
            wb = wbpool.tile([128, 1024], BF16, tag="Wqb")
            nc.vector.tensor_copy(wb[:], w[:])
            wts.append(wb)
        for jj in range(8):
            j = jg * 8 + jj
            h = j // KC
            kc = j % KC
            ps = psum.tile([128, BL], F32, tag="sm")
            for k in range(KC):
                nc.tensor.matmul(ps[:], wts[k][:, jj * 128:(jj + 1) * 128],
                                 qcT[k][:], start=(k == 0),
                                 stop=(k == KC - 1), skip_group_check=True)
            base = kc * 512 + h
            nc.vector.tensor_scalar(
                out=Qpad[:, base:base + (BL - 1) * 72 + 1:72],
                in0=ps[:], scalar1=bq[:, j:j + 1],
                scalar2=None, op0=OP.add)

    # -------- Wagg: stream early, cast to bf16 on idle GpSimd, residents --
    waggb = []
    for g in range(16):
        wstg = wpool.tile([128, 2, VD], F32, tag="Waggstg")
        engs2 = [nc.sync, nc.scalar, nc.gpsimd]
        engs2[g % 3].dma_start(wstg[:], io["Wagg"][g * 256:(g + 1) * 256, :]
                      .rearrange("(f p) c -> p f c", p=128))
        wgb = wrpool.tile([128, 2, VD], BF16, tag="Waggb")
        nc.gpsimd.tensor_copy(wgb[:], wstg[:])
        waggb.append(wgb)

    # ---------------- Phase B: keys (pre-transposed [K, B, L]) + scores ----
    # Keys arrive in [k, b, l] layout (relayout chosen at shard time), so
    # each [128, 4, 1024] f32 DMA slice is cast to bf16 and fed straight to
    # the PE as the moving operand.  Zero-padded lhsT -> every matmul
    # writes the full [64, 512] bank; one accumulation group per lh bank
    # spans all (kc, b).
    S = pool.tile([B, L], F32)
    sp_half0 = spsum.tile([B, 512], F32, tag="sp")
    sp_half1 = spsum.tile([B, 512], F32, tag="sp")
    sp_halves = [sp_half0, sp_half1]
    engs = [nc.sync, nc.scalar, nc.gpsimd]
    for kc in range(KC):
        for q in range(4):
            i = kc * 4 + q
            b0 = q * 2
            ktn = kpool.tile([128, 2, L], F32, tag="ktn")
            engs[i % 3].dma_start(
                ktn[:], io["keysT"][kc * 128:(kc + 1) * 128, b0:b0 + 2, :])
            ktb = kbpool.tile([128, 2, L], BF16, tag="ktb")
            if i % 2 == 0:
                nc.vector.tensor_copy(ktb[:], ktn[:])
            else:
                nc.scalar.copy(ktb[:], ktn[:])
            for bl in range(2):
                b = b0 + bl
                for lh in range(2):
                    nc.tensor.matmul(sp_halves[lh][:],
                                     Qpad[:, kc * 512 + b * 64:
                                          kc * 512 + (b + 1) * 64],
                                     ktb[:, bl, lh * 512:(lh + 1) * 512],
                                     start=(kc == 0 and q == 0 and bl == 0),
                                     stop=(kc == KC - 1 and q == 3
                                           and bl == 1),
                                     skip_group_check=True)
    for lh in range(2):
        nc.vector.tensor_copy(S[:, lh * 512:(lh + 1) * 512], sp_halves[lh][:])

    # ---------------- Phase C: mask + softmax ------------------------------
    iot = pool.tile([B, L], F32)
    nc.gpsimd.iota(iot[:], pattern=[[1, L]], base=0, channel_multiplier=0,
                   allow_small_or_imprecise_dtypes=True)
    stept = pool.tile([B, 1], F32)
    nc.sync.dma_start(stept[:], io["step_rep"][:])
    valid = pool.tile([B, L], F32)
    nc.vector.tensor_scalar(out=valid[:], in0=iot[:], scalar1=stept[:, 0:1],
                            scalar2=None, op0=OP.is_lt)
    A = pool.tile([B, L], F32, tag="iot")
    nc.scalar.activation(A[:], valid[:], AF.Copy, bias=-1e30, scale=1e30)

    rpeT = pool.tile([BL, L], F32)
    for lc in range(LC):
        rp = pool.tile([128, BL], F32, tag="rp")
        nc.sync.dma_start(rp[:], io["rpe"][lc * 128:(lc + 1) * 128, :])
        tp = psum.tile([BL, 128], F32, tag="sm")
        nc.tensor.transpose(tp[:], rp[:], ident[:])
        nc.vector.tensor_copy(rpeT[:, lc * 128:(lc + 1) * 128], tp[:])
    selt = pool.tile([BL, B], F32)
    nc.sync.dma_start(selt[:], io["sel"][:])
    G = pool.tile([B, L], F32)
    for lh in range(2):
        gp = spsum.tile([B, 512], F32, tag="sp")
        nc.tensor.matmul(gp[:], selt[:], rpeT[:, lh * 512:(lh + 1) * 512],
                         start=True, stop=True)
        nc.vector.tensor_tensor(out=G[:, lh * 512:(lh + 1) * 512], in0=gp[:],
                                in1=valid[:, lh * 512:(lh + 1) * 512],
                                op=OP.mult)

    nc.vector.tensor_tensor(out=S[:], in0=S[:], in1=G[:], op=OP.mult)
    nc.vector.tensor_tensor(out=S[:], in0=S[:], in1=A[:], op=OP.add)
    negM = pool.tile([B, 1], F32)
    nc.vector.tensor_reduce(out=negM[:], in_=S[:], op=OP.max,
                            axis=mybir.AxisListType.X, negate=True)
    E = pool.tile([B, L], F32, tag="G")
    Z = pool.tile([B, 1], F32)
    nc.scalar.activation(E[:], S[:], AF.Exp, bias=negM[:, 0:1], scale=1.0,
                         accum_out=Z[:, 0:1])
    R = pool.tile([B, 1], F32)
    nc.vector.reciprocal(R[:], Z[:])
    P = pool.tile([B, L], BF16, tag="rpeT")
    nc.vector.tensor_scalar(out=P[:], in0=E[:], scalar1=R[:, 0:1],
                            scalar2=None, op0=OP.mult)

    # ---------------- Phase D: prob transpose + value matmul ---------------
    PTs = []
    for lc in range(LC):
        PT = pool.tile([128, B], BF16, tag=f"PT{lc}")
        tpp = psum.tile([128, B], BF16, tag="sm")
        nc.tensor.transpose(tpp[:], P[:, lc * 128:(lc + 1) * 128],
                            identb[0:B, 0:B])
        nc.vector.tensor_copy(PT[:], tpp[:])
        PTs.append(PT)

    T = pool.tile([128, VD // 128, H, BL], BF16)
    for b in range(BL):
        rps = spsum.tile([BL, VD], F32, tag="sp")
        for lq in range(LC // 2):
            i = b * (LC // 2) + lq
            vn = vpool.tile([128, 2, VD], F32, tag="vnat")
            veng = engs[i % 3]
            veng.dma_start(
                vn[:], io["vals"][lq * 256:(lq + 1) * 256, b, :]
                .rearrange("(f p) c -> p f c", p=128))
            vb = vbpool.tile([128, 2, VD], BF16, tag="vb")
            if i % 2 == 0:
                nc.vector.tensor_copy(vb[:], vn[:])
            else:
                nc.scalar.copy(vb[:], vn[:])
            for f in range(2):
                lc = lq * 2 + f
                nc.tensor.matmul(rps[:], PTs[lc][:, b * H:(b + 1) * H],
                                 vb[:, f, :],
                                 start=(lc == 0), stop=(lc == LC - 1),
                                 skip_group_check=True)
        rs = pool.tile([BL, VD], BF16, tag="rs")
        nc.scalar.copy(rs[:], rps[:])
        for vs in range(VD // 128):
            tr = psum.tile([128, BL], BF16, tag="sm")
            nc.tensor.transpose(tr[:], rs[:, vs * 128:(vs + 1) * 128],
                                identb[0:BL, 0:BL])
            nc.vector.tensor_copy(T[:, vs, :, b], tr[:])

    # ---------------- Phase E: output MLP chain ----------------------------
    bagg = bias_tile("bagg", 4)
    brk1 = bias_tile("brk1", 4)
    brv1 = bias_tile("brv1", 4)

    aggp = spsum.tile([BL, VD], F32, tag="sp")
    for c in range(32):
        g, f = c // 4, c % 4
        h = c // (VD // 128)
        vs = c % (VD // 128)
        nc.tensor.matmul(aggp[:], T[:, vs, h, :], waggb[c // 2][:, c % 2, :],
                         start=(c == 0), stop=(c == 31),
                         skip_group_check=True)
    AT = nat_to_T(aggp, VD, bagg, "AT")

    # final-layer biases broadcast to [8, 512] via K=1 matmul
    ones = pool.tile([1, BL], F32)
    nc.gpsimd.memset(ones[:], 1.0)

    def bias_bcast(name):
        brow = pool.tile([1, 512], F32, tag="br" + name)
        nc.sync.dma_start(brow[:], io[name][:])
        bb = psum.tile([BL, 512], F32, tag="sm")
        nc.tensor.matmul(bb[:], ones[:], brow[:], start=True, stop=True)
        bsb = pool.tile([BL, 512], F32, tag="bs" + name)
        nc.vector.tensor_copy(bsb[:], bb[:])
        return bsb

    bk2 = bias_bcast("brk2_flat")
    bv2 = bias_bcast("brv2_flat")

    hkT = layer_T(AT, "Wrk1", brk1, HID, "hk")
    ok_ps = layer_bf16(hkT, "Wrk2", RIMQ)
    hvT = layer_T(AT, "Wrv1", brv1, HID, "hv", eng=nc.scalar)
    ov_ps = layer_bf16(hvT, "Wrv2", VD, eng=nc.scalar)

    for name, ps_, bias_sb in (("out_key", ok_ps, bk2), ("out_val", ov_ps, bv2)):
        onat = pool.tile([BL, 512], F32, tag="o" + name)
        nc.vector.tensor_tensor(out=onat[:], in0=ps_[:], in1=bias_sb[:],
                                op=OP.add)
        nc.sync.dma_start(io[name][:], onat[:])


def _build():
    nc = bacc.Bacc("TRN2", target_bir_lowering=False, debug=False,
                   num_devices=NCORES)
    io = {}

    def din(name, shape):
        io[name] = nc.dram_tensor(name, shape, F32, kind="ExternalInput").ap()

    din("keysT", [KD, BL, L])
    din("vals", [L, BL, VD])
    din("rpe", [L, BL])
    din("step_rep", [B, 1])
    din("state", [BL, SDIM])
    din("lat", [BL, LAT])
    din("sel", [BL, B])
    din("W_state", [SDIM, MEMB])
    din("b_state", [128, 2])
    din("Wcq1", [KD, HID])
    din("bcq1", [128, 4])
    din("Wcq2", [HID, KD])
    din("bcq2", [128, 4])
    din("Wq", [KD, H * KD])
    din("bq", [128, 32])
    din("Wagg", [H * VD, VD])
    din("bagg", [128, 4])
    din("Wrk1", [VD, HID])
    din("brk1", [128, 4])
    din("Wrk2", [HID, RIMQ])
    din("brk2_flat", [1, 512])
    din("Wrv1", [VD, HID])
    din("brv1", [128, 4])
    din("Wrv2", [HID, VD])
    din("brv2_flat", [1, 512])
    io["out_key"] = nc.dram_tensor("out_key", [BL, RIMQ], F32,
                                   kind="ExternalOutput").ap()
    io["out_val"] = nc.dram_tensor("out_val", [BL, VD], F32,
                                   kind="ExternalOutput").ap()

    with tile.TileContext(nc) as tc, ExitStack() as ctx:
        _emit(nc, tc, ctx, io)
    nc.compile()
    return nc


def _rsb(bias, nch):
    return np.ascontiguousarray(
        np.asarray(bias, np.float32).reshape(nch, 128).T)


def _shard(inputs):
    f = lambda x: np.asarray(x, np.float32)
    keys, vals, rpe = f(inputs["keys"]), f(inputs["vals"]), f(inputs["rpe_mod"])
    step = np.asarray(inputs["step"]).astype(np.float32)
    state, lat = f(inputs["state"]), f(inputs["task_inference_latent"])
    sel = np.ascontiguousarray(
        np.repeat(np.eye(BL, dtype=np.float32), BL, axis=1) * RSQK)
    shared = {
        "sel": sel,
        "W_state": f(inputs["W_state"]), "b_state": _rsb(inputs["b_state"], 2),
        "Wcq1": f(inputs["Wcq1"]), "bcq1": _rsb(inputs["bcq1"], 4),
        "Wcq2": f(inputs["Wcq2"]), "bcq2": _rsb(inputs["bcq2"], 4),
        "Wq": f(inputs["Wq"]), "bq": _rsb(inputs["bq"], 32),
        "Wagg": f(inputs["Wagg"]), "bagg": _rsb(inputs["bagg"], 4),
        "Wrk1": f(inputs["Wrk1"]), "brk1": _rsb(inputs["brk1"], 4),
        "Wrk2": f(inputs["Wrk2"]),
        "brk2_flat": np.ascontiguousarray(f(inputs["brk2"])[None, :]),
        "Wrv1": f(inputs["Wrv1"]), "brv1": _rsb(inputs["brv1"], 4),
        "Wrv2": f(inputs["Wrv2"]),
        "brv2_flat": np.ascontiguousarray(f(inputs["brv2"])[None, :]),
    }
    in_maps = []
    for m in range(NCORES):
        b0 = m * BL
        in_maps.append({
            "keysT": np.ascontiguousarray(
                keys[:, b0:b0 + BL, :].transpose(2, 1, 0)),
            "vals": np.ascontiguousarray(vals[:, b0:b0 + BL, :]),
            "rpe": np.ascontiguousarray(rpe[:, b0:b0 + BL, 0]),
            "step_rep": np.ascontiguousarray(
                np.repeat(step[b0:b0 + BL], H)[:, None]),
            "state": np.ascontiguousarray(state[b0:b0 + BL]),
            "lat": np.ascontiguousarray(lat[b0:b0 + BL]),
            **shared,
        })
    return in_maps


def kernel(**inputs):
    nc = _CACHE.get("nc")
    if nc is None:
        nc = _CACHE["nc"] = _build()
    in_maps = _shard(inputs)
    res = run_bass_kernel_spmd(nc, in_maps, list(range(NCORES)),
                               **_CACHE.get("run_kwargs", {}))
    _CACHE["last_result"] = res
    ok = np.concatenate([res.results[m]["out_key"] for m in range(NCORES)], 0)
    ov = np.concatenate([res.results[m]["out_val"] for m in range(NCORES)], 0)
    return ok[:, None, :], ov[:, None, :]
